# revision 1
# baseline (speedup 1.0000x reference)
"""Trainium2 Bass kernel for MinecraftAwareAttention3D.

Full (unsharded) inputs in, full output out. Internally: one attention head
per NeuronCore (tensor parallel over num_heads=8), GroupNorm + QKV replicated
per core, per-head projection partials summed on the host.

Key tricks:
  * The material / vertical boosts are bilinear in small per-position feature
    vectors, so they are folded into the QK^T matmul as 18 extra contraction
    rows (Q' = [q*scale; L], K' = [k; R] -> S = Q'.K' = qk*scale + boost).
  * Air keys are compacted away on the host (softmax weight for them is
    exactly 0), shrinking the key axis from 4096 to ~3072.
  * No max-subtraction in softmax (logits are O(10), exp cannot overflow);
    exp of padded key rows is killed with a -1e9 per-partition ACT bias.
  * A ones-row appended to V^T makes the PV matmul also produce the softmax
    denominator.
  * float32r (full-rate fp32 matmul mode) everywhere on the PE.
"""

import numpy as np

import concourse.bass as bass
import concourse.tile as tile
from concourse import mybir
from concourse.bass_utils import run_bass_kernel_spmd

F32 = mybir.dt.float32
F32R = mybir.dt.float32r
AF = mybir.ActivationFunctionType
ALU = mybir.AluOpType

B, C, D_, H_, W_ = 1, 256, 16, 16, 16
N = D_ * H_ * W_          # 4096 spatial positions
HEADS, HD = 8, 32
GROUPS = 8                # groupnorm groups -> 32 channels per group
GSIZE = C // GROUPS
EPS = 1e-5
NEG = -1e9
NF = HD + 18              # fused contraction depth: 32 qk dims + 18 boost dims
NCORES = 8

TRACE = False             # test.py can flip this for profiling
LAST_RESULT = {}

_CACHE = {}


def _split_waits(nc, max_waits=1):
    """This walrus build only encodes one sync wait per instruction; hoist
    extra waits onto same-engine NOPs inserted just before the instruction."""
    n = 0
    for f in nc.m.functions:
        for bb in f.blocks:
            new_insts = []
            for inst in bb.instructions:
                si = inst.sync_info
                if si is not None and si.on_wait and len(si.on_wait) > max_waits:
                    waits = list(si.on_wait)
                    si.on_wait = waits[-max_waits:]
                    for i in range(0, len(waits) - max_waits, max_waits):
                        n += 1
                        nop = mybir.InstNoOp(name=f"I-wsplit-{n}", ins=[], outs=[])
                        nop.engine = inst.engine
                        nop.sync_info = mybir.SyncInfo(
                            on_wait=waits[i : i + max_waits], on_update=[]
                        )
                        new_insts.append(nop)
                new_insts.append(inst)
            bb.instructions[:] = new_insts
    return n


def _build(nk_pad):
    """Build the per-core Bass module. Static on the padded compacted key
    count; all data arrives as ExternalInputs."""
    nkc = nk_pad // 128                       # 128-key chunks
    ksl = [min(512, nk_pad - s) for s in range(0, nk_pad, 512)]  # kv col chunks

    nc = bass.Bass()

    # ---- I/O ----
    x2 = nc.dram_tensor("x2", [C, N], F32, kind="ExternalInput")
    xc = nc.dram_tensor("xc", [C, nk_pad], F32, kind="ExternalInput")
    lfeat = nc.dram_tensor("lfeat", [18, N], F32R, kind="ExternalInput")
    rfeat = nc.dram_tensor("rfeat", [18, nk_pad], F32R, kind="ExternalInput")
    abias = nc.dram_tensor("abias", [128, nkc], F32, kind="ExternalInput")
    gseli = nc.dram_tensor("gseli", [128, 4], F32R, kind="ExternalInput")
    gselt = nc.dram_tensor("gselt", [4, 128], F32R, kind="ExternalInput")
    ident = nc.dram_tensor("ident", [33, 33], F32, kind="ExternalInput")
    one32 = nc.dram_tensor("one32", [1, HD], F32R, kind="ExternalInput")
    onesk = nc.dram_tensor("onesk", [1, nk_pad], F32, kind="ExternalInput")
    gnw = nc.dram_tensor("gnw", [C, 1], F32, kind="ExternalInput")
    gnb = nc.dram_tensor("gnb", [C, 1], F32, kind="ExternalInput")
    wq = nc.dram_tensor("wq", [C, HD], F32R, kind="ExternalInput")
    wk = nc.dram_tensor("wk", [C, HD], F32R, kind="ExternalInput")
    wv = nc.dram_tensor("wv", [C, HD], F32R, kind="ExternalInput")
    bq = nc.dram_tensor("bq", [HD, 1], F32, kind="ExternalInput")
    bk = nc.dram_tensor("bk", [HD, 1], F32, kind="ExternalInput")
    bv = nc.dram_tensor("bv", [HD, 1], F32, kind="ExternalInput")
    pwt = nc.dram_tensor("pwt", [HD, C], F32R, kind="ExternalInput")
    out = nc.dram_tensor("o", [C, N], F32, kind="ExternalOutput")

    with tile.TileContext(nc) as tc:
        with (
            tc.tile_pool(name="consts", bufs=1) as cp,
            tc.tile_pool(name="live", bufs=1) as lp,
            tc.tile_pool(name="small", bufs=2) as sp,
            tc.tile_pool(name="ps_small", bufs=2, space="PSUM") as pss,
            tc.tile_pool(name="ps_s", bufs=2, space="PSUM") as ps_s,
            tc.tile_pool(name="ps_pv", bufs=1, space="PSUM") as ps_pv,
        ):
            # ---- long-lived activations ----
            h = [lp.tile([128, N], F32R, name=f"h{c}") for c in range(2)]
            hk = [lp.tile([128, nk_pad], F32R, name=f"hk{c}") for c in range(2)]
            qf = lp.tile([NF, N], F32R)           # Q' = [q*scale ; L]
            kf = lp.tile([NF, nk_pad], F32R)      # K' = [k ; R]
            vv = lp.tile([HD + 1, nk_pad], F32)   # [v ; ones]
            vt = lp.tile([128, nkc, HD + 1], F32R)  # per-chunk V'^T

            # Warm the ACT exp table-set before anything else touches ACT:
            # every later ACT op (Identity/Copy adds, Exp) then runs from the
            # already-resident set with no mid-kernel table switch.
            wz = cp.tile([1, 1], F32)
            nc.vector.memset(wz, 0.0)
            wy = cp.tile([1, 1], F32)
            nc.scalar.activation(out=wy, in_=wz, func=AF.Exp, bias=0.0, scale=1.0)

            # ================= Phase 1: GroupNorm =================
            with tc.tile_pool(name="xpool", bufs=1) as xp:
                # x loads go first on the HWDGE queues: they gate the whole
                # GroupNorm -> QKV -> attention chain.
                xt = [xp.tile([128, N], F32, name=f"xt{c}") for c in range(2)]
                for c in range(2):
                    for s in range(4):
                        nc.sync.dma_start(
                            out=xt[c][:, s * 1024 : (s + 1) * 1024],
                            in_=x2[c * 128 : (c + 1) * 128, s * 1024 : (s + 1) * 1024],
                        )

                # ---- constants (SWDGE queues, off the critical path) ----
                gsel_t = cp.tile([128, 4], F32R)
                nc.gpsimd.dma_start(out=gsel_t, in_=gseli[:, :])
                gselt_t = cp.tile([4, 128], F32R)
                nc.gpsimd.dma_start(out=gselt_t, in_=gselt[:, :])
                ident_t = cp.tile([33, 33], F32)
                nc.gpsimd.dma_start(out=ident_t, in_=ident[:, :])
                gnw_t = cp.tile([C // 2, 2], F32)
                nc.gpsimd.dma_start(out=gnw_t[:, 0:1], in_=gnw[0:128, :])
                nc.gpsimd.dma_start(out=gnw_t[:, 1:2], in_=gnw[128:256, :])
                gnb_t = cp.tile([C // 2, 2], F32)
                nc.gpsimd.dma_start(out=gnb_t[:, 0:1], in_=gnb[0:128, :])
                nc.gpsimd.dma_start(out=gnb_t[:, 1:2], in_=gnb[128:256, :])
                wq_t = cp.tile([128, 2, HD], F32R)
                wk_t = cp.tile([128, 2, HD], F32R)
                wv_t = cp.tile([128, 2, HD], F32R)
                for t, d in ((wq_t, wq), (wk_t, wk), (wv_t, wv)):
                    nc.gpsimd.dma_start(out=t[:, 0, :], in_=d[0:128, :])
                    nc.gpsimd.dma_start(out=t[:, 1, :], in_=d[128:256, :])
                bq_t = cp.tile([HD, 1], F32)
                nc.gpsimd.dma_start(out=bq_t, in_=bq[:, :])
                bk_t = cp.tile([HD, 1], F32)
                nc.gpsimd.dma_start(out=bk_t, in_=bk[:, :])
                bv_t = cp.tile([HD, 1], F32)
                nc.gpsimd.dma_start(out=bv_t, in_=bv[:, :])
                pwt_t = cp.tile([HD, C], F32R)
                nc.gpsimd.dma_start(out=pwt_t, in_=pwt[:, :])
                abias_t = cp.tile([128, nkc], F32)
                nc.gpsimd.dma_start(out=abias_t, in_=abias[:, :])
                ones_t = cp.tile([1, HD], F32R)
                nc.gpsimd.dma_start(out=ones_t, in_=one32[:, :])
                nc.gpsimd.dma_start(out=qf[HD:NF, :], in_=lfeat[:, :])
                nc.gpsimd.dma_start(out=kf[HD:NF, :], in_=rfeat[:, :])
                nc.gpsimd.dma_start(out=vv[HD : HD + 1, :], in_=onesk[:, :])

                stats2 = sp.tile([128, 4], F32R)  # [mean_c0, E2_c0, mean_c1, E2_c1]
                for c in range(2):
                    st6 = sp.tile([128, 8, 6], F32, name=f"st6_{c}")
                    for s in range(8):
                        nc.vector.bn_stats(
                            out=st6[:, s, :], in_=xt[c][:, s * 512 : (s + 1) * 512]
                        )
                    mv = sp.tile([128, 2], F32, name=f"mv_{c}")
                    nc.vector.bn_aggr(out=mv, in_=st6)
                    m2 = sp.tile([128, 1], F32, name=f"m2_{c}")
                    nc.vector.tensor_mul(out=m2, in0=mv[:, 0:1], in1=mv[:, 0:1])
                    nc.vector.tensor_copy(
                        out=stats2[:, 2 * c : 2 * c + 1], in_=mv[:, 0:1]
                    )
                    nc.vector.tensor_add(
                        out=stats2[:, 2 * c + 1 : 2 * c + 2], in0=mv[:, 1:2], in1=m2
                    )

                gstat = pss.tile([4, 4], F32, space="PSUM", tag="s")
                nc.tensor.matmul(gstat, lhsT=gsel_t, rhs=stats2, start=True, stop=True)

                ab = []
                for c in range(2):
                    mu4 = sp.tile([4, 1], F32R, name=f"mu4_{c}")
                    nc.vector.tensor_scalar_mul(
                        out=mu4, in0=gstat[:, 2 * c : 2 * c + 1], scalar1=1.0 / GSIZE
                    )
                    ve = sp.tile([4, 1], F32, name=f"ve_{c}")
                    nc.vector.tensor_scalar_mul(
                        out=ve, in0=gstat[:, 2 * c + 1 : 2 * c + 2], scalar1=1.0 / GSIZE
                    )
                    mum = sp.tile([4, 1], F32, name=f"mum_{c}")
                    nc.vector.tensor_mul(out=mum, in0=mu4, in1=mu4)
                    nc.vector.tensor_sub(out=ve, in0=ve, in1=mum)
                    nc.vector.tensor_scalar_add(out=ve, in0=ve, scalar1=EPS)
                    # rsqrt without ACT (avoids the sqrt table-set load):
                    # quake seed y = bits(0x5f3759df - (i >> 1)), then 3
                    # Newton steps y' = y * (1.5 - 0.5*v*y^2)
                    I32 = mybir.dt.int32
                    yi = sp.tile([4, 1], I32, name=f"yi_{c}")
                    nc.vector.tensor_scalar(
                        out=yi, in0=ve.bitcast(I32), scalar1=1, scalar2=None,
                        op0=ALU.logical_shift_right,
                    )
                    nc.vector.tensor_scalar(
                        out=yi, in0=yi, scalar1=-1, scalar2=0x5F3759DF,
                        op0=ALU.mult, op1=ALU.add,
                    )
                    rs = sp.tile([4, 1], F32, name=f"rs_{c}")
                    nc.vector.tensor_copy(out=rs, in_=yi.bitcast(F32))
                    t2 = sp.tile([4, 1], F32, name=f"t2_{c}")
                    for it in range(3):
                        nc.vector.tensor_mul(out=t2, in0=rs, in1=rs)
                        nc.vector.tensor_mul(out=t2, in0=t2, in1=ve)
                        nc.vector.tensor_scalar(
                            out=t2, in0=t2, scalar1=-0.5, scalar2=1.5,
                            op0=ALU.mult, op1=ALU.add,
                        )
                        nc.vector.tensor_mul(out=rs, in0=rs, in1=t2)
                    rs2 = sp.tile([4, 1], F32R, name=f"rs2_{c}")
                    nc.vector.tensor_copy(out=rs2, in_=rs)

                    musig = sp.tile([4, 2], F32R, name=f"musig_{c}")
                    nc.vector.tensor_copy(out=musig[:, 0:1], in_=mu4)
                    nc.vector.tensor_copy(out=musig[:, 1:2], in_=rs2)
                    bc = pss.tile([128, 2], F32, space="PSUM", name=f"bc_{c}", tag="s")
                    nc.tensor.matmul(bc, lhsT=gselt_t, rhs=musig, start=True, stop=True)
                    # a = gn_w * rstd ; b = gn_b - mu * a
                    a_ch = sp.tile([128, 1], F32, name=f"a_ch_{c}")
                    nc.vector.tensor_mul(out=a_ch, in0=gnw_t[:, c : c + 1], in1=bc[:, 1:2])
                    b_ch = sp.tile([128, 1], F32, name=f"b_ch_{c}")
                    nc.vector.tensor_mul(out=b_ch, in0=bc[:, 0:1], in1=a_ch)
                    nc.vector.tensor_sub(out=b_ch, in0=gnb_t[:, c : c + 1], in1=b_ch)
                    ab.append((a_ch, b_ch))

                    # first query half only: unblocks q-matmuls for the
                    # first two q-groups; second half follows off-path
                    nc.vector.tensor_scalar(
                        out=h[c][:, 0 : N // 2], in0=xt[c][:, 0 : N // 2],
                        scalar1=a_ch, scalar2=b_ch, op0=ALU.mult, op1=ALU.add,
                    )
                # compacted keys: stream xc and normalize into hk
                # (slice-major so early key slices finish first)
                for s0 in range(0, nk_pad, 1024):
                    s1 = min(s0 + 1024, nk_pad)
                    for c in range(2):
                        a_ch, b_ch = ab[c]
                        xs_t = xp.tile([128, 1024], F32, name="xcs", tag="xcs", bufs=4)
                        nc.sync.dma_start(
                            out=xs_t[:, 0 : s1 - s0],
                            in_=xc[c * 128 : (c + 1) * 128, s0:s1],
                        )
                        nc.vector.tensor_scalar(
                            out=hk[c][:, s0:s1], in0=xs_t[:, 0 : s1 - s0],
                            scalar1=a_ch, scalar2=b_ch, op0=ALU.mult, op1=ALU.add,
                        )
                for c in range(2):
                    a_ch, b_ch = ab[c]
                    nc.vector.tensor_scalar(
                        out=h[c][:, N // 2 :], in0=xt[c][:, N // 2 :],
                        scalar1=a_ch, scalar2=b_ch, op0=ALU.mult, op1=ALU.add,
                    )

            with (
                tc.tile_pool(name="pbig", bufs=3) as pb,
                tc.tile_pool(name="opool", bufs=3) as op,
            ):
                # ================= Phase 2: QKV + V transpose =================
                ksn = len(ksl)
                for i in range(max(8, ksn)):
                    paths = []
                    if i < ksn:
                        paths.append((wk_t, bk_t, kf, hk, ksl[i], i * 512))
                    if i < 8:
                        paths.append((wq_t, bq_t, qf, h, 512, i * 512))
                    if i < ksn:
                        paths.append((wv_t, bv_t, vv, hk, ksl[i], i * 512))
                    for j, (w_t, b_t, dst, src_t, w_n, s0) in enumerate(paths):
                        sl = slice(s0, s0 + w_n)
                        # borrow the (idle) attention PSUM slots for extra
                        # buffering so matmuls are not slot-gated on copies
                        pool = ps_s if j % 2 == 0 else pss
                        tag = "st" if j % 2 == 0 else "s"
                        ps = pool.tile([HD, 512], F32, space="PSUM", name="qkv_ps", tag=tag)
                        nc.tensor.matmul(
                            ps[:, 0:w_n], lhsT=w_t[:, 0, :], rhs=src_t[0][:, sl],
                            start=True, stop=False,
                        )
                        nc.tensor.matmul(
                            ps[:, 0:w_n], lhsT=w_t[:, 1, :], rhs=src_t[1][:, sl],
                            start=False, stop=True,
                        )
                        if dst is kf:  # k on DVE; q/v on the idle ACT engine
                            nc.vector.tensor_scalar_add(
                                out=dst[0:HD, sl], in0=ps[:, 0:w_n], scalar1=b_t
                            )
                        else:
                            nc.scalar.add(
                                out=dst[0:HD, sl], in_=ps[:, 0:w_n], add=b_t
                            )

                for kc in range(nkc):
                    tps = pss.tile([128, HD + 1], F32, space="PSUM", name="tr_ps", tag="s")
                    nc.tensor.transpose(
                        tps, in_=vv[:, kc * 128 : (kc + 1) * 128], identity=ident_t
                    )
                    nc.vector.tensor_copy(out=vt[:, kc, :], in_=tps)

                # ========== Phase 3: attention (+ fused projection) ==========
                for qg in range(4):
                    q0 = qg * 1024
                    pv = ps_pv.tile([HD + 1, 1024], F32, space="PSUM", name="pv")
                    for kc in range(nkc):
                        st = ps_s.tile([128, 1024], F32, space="PSUM", name="st")
                        lhs = kf[:, kc * 128 : (kc + 1) * 128]
                        nc.tensor.matmul(
                            st[:, 0:512], lhsT=lhs, rhs=qf[:, q0 : q0 + 512],
                            start=True, stop=True,
                        )
                        nc.tensor.matmul(
                            st[:, 512:1024], lhsT=lhs, rhs=qf[:, q0 + 512 : q0 + 1024],
                            start=True, stop=True,
                        )
                        pt = pb.tile([128, 1024], F32R, name="pt")
                        nc.scalar.activation(
                            out=pt, in_=st, func=AF.Exp,
                            bias=abias_t[:, kc : kc + 1], scale=1.0,
                        )
                        vlhs = vt[:, kc, :]
                        nc.tensor.matmul(
                            pv[:, 0:512], lhsT=vlhs, rhs=pt[:, 0:512],
                            start=(kc == 0), stop=(kc == nkc - 1), skip_group_check=True,
                        )
                        nc.tensor.matmul(
                            pv[:, 512:1024], lhsT=vlhs, rhs=pt[:, 512:1024],
                            start=(kc == 0), stop=(kc == nkc - 1), skip_group_check=True,
                        )
                    # evacuate the accumulator to free the PSUM banks.
                    # high_priority pulls the whole normalize+project+store
                    # chain ahead of the next q-group's matmuls so the output
                    # DMA streams during attention instead of at the tail.
                    hp = tc.high_priority()
                    hp.__enter__()
                    ohu = op.tile([HD, 1024], F32, name="ohu", tag="ohu", bufs=2)
                    nc.vector.tensor_copy(out=ohu, in_=pv[0:HD, :])
                    r2 = sp.tile([1, 1024], F32R, name="r2", tag="rr", bufs=3)
                    with nc.allow_low_precision(reason="f32r reciprocal for PE broadcast"):
                        nc.vector.reciprocal(out=r2, in_=pv[HD : HD + 1, :])
                    ohn = op.tile([HD, 1024], F32R, name="ohn", tag="ohn", bufs=2)
                    for hf in range(2):
                        sl = slice(hf * 512, (hf + 1) * 512)
                        rbc = pss.tile([HD, 512], F32, space="PSUM", name="rbc", tag="s")
                        nc.tensor.matmul(rbc, lhsT=ones_t, rhs=r2[:, sl], start=True, stop=True)
                        nc.vector.tensor_mul(out=ohn[:, sl], in0=ohu[:, sl], in1=rbc)
                    hp.__exit__(None, None, None)
                    for hf in range(2):
                        sl = slice(hf * 512, (hf + 1) * 512)
                        osl = slice(q0 + hf * 512, q0 + (hf + 1) * 512)
                        for c in range(2):
                            pp = pss.tile([128, 512], F32, space="PSUM", name="pp", tag="s")
                            nc.tensor.matmul(
                                pp, lhsT=pwt_t[:, c * 128 : (c + 1) * 128], rhs=ohn[:, sl],
                                start=True, stop=True,
                            )
                            ot = op.tile([128, 512], F32, name="ot")
                            nc.vector.tensor_copy(out=ot, in_=pp)
                            nc.sync.dma_start(
                                out=out[c * 128 : (c + 1) * 128, osl], in_=ot
                            )

    _split_waits(nc)
    return nc


def _numpy_reference(x, block_types, gn_w, gn_b, qkv_w, qkv_b, proj_w, proj_b,
                     is_air, is_wood, is_leaves):
    """Pure-numpy fallback (degenerate case: no non-air keys)."""
    xf = x.reshape(B, C, N).astype(np.float64)
    xs = xf.reshape(B, GROUPS, GSIZE * N)
    mu = xs.mean(axis=2, keepdims=True)
    var = xs.var(axis=2, keepdims=True)
    h = ((xs - mu) / np.sqrt(var + EPS)).reshape(B, C, N)
    h = h * gn_w[None, :, None] + gn_b[None, :, None]
    qkv = np.einsum("oc,bcn->bon", qkv_w.astype(np.float64), h) + qkv_b[None, :, None]
    qkv = qkv.reshape(B, 3, HEADS, HD, N)
    q, k, v = qkv[:, 0], qkv[:, 1], qkv[:, 2]
    attn = np.einsum("bhdn,bhdm->bhnm", q, k) * (HD ** -0.5)
    bf = block_types.reshape(B, N)
    air = is_air[bf]; wood = is_wood[bf]; leaves = is_leaves[bf]
    attn = np.where(air[:, None, None, :] > 0, NEG, attn)
    wo = wood[:, :, None] * wood[:, None, :]
    lo = leaves[:, :, None] * leaves[:, None, :]
    mb = np.clip((wo + lo) * 2.0, 0.0, 10.0)
    pos = np.arange(N); ypos = (pos // W_) % H_
    vm = (np.abs(ypos[None, :] - ypos[:, None]) <= 2).astype(np.float64)
    vb = np.clip(wo * vm[None] * 1.5, 0.0, 10.0)
    attn = attn + (mb + vb)[:, None]
    attn = attn - attn.max(axis=-1, keepdims=True)
    e = np.exp(attn); p = e / e.sum(axis=-1, keepdims=True)
    o = np.einsum("bhnm,bhdm->bhdn", p, v).reshape(B, C, N)
    o = np.einsum("oc,bcn->bon", proj_w.astype(np.float64), o) + proj_b[None, :, None]
    return (xf + o).reshape(x.shape).astype(np.float32)


def kernel(x, block_types, gn_w, gn_b, qkv_w, qkv_b, proj_w, proj_b,
           is_air, is_wood, is_leaves):
    x = np.ascontiguousarray(np.asarray(x, dtype=np.float32))
    gn_w = np.asarray(gn_w, np.float32); gn_b = np.asarray(gn_b, np.float32)
    qkv_w = np.asarray(qkv_w, np.float32); qkv_b = np.asarray(qkv_b, np.float32)
    proj_w = np.asarray(proj_w, np.float32); proj_b = np.asarray(proj_b, np.float32)
    is_air = np.asarray(is_air, np.float32)
    is_wood = np.asarray(is_wood, np.float32)
    is_leaves = np.asarray(is_leaves, np.float32)
    bt = np.asarray(block_types).reshape(N).astype(np.int64)

    x2 = x.reshape(C, N)
    air = is_air[bt]; wood = is_wood[bt]; leaves = is_leaves[bt]
    idx = np.nonzero(air <= 0.0)[0]
    nk = len(idx)
    if nk == 0:
        return _numpy_reference(x, block_types, gn_w, gn_b, qkv_w, qkv_b,
                                proj_w, proj_b, is_air, is_wood, is_leaves)

    nk_pad = ((nk + 127) // 128) * 128
    nkc = nk_pad // 128
    idx_pad = np.concatenate([idx, np.full(nk_pad - nk, idx[0], np.int64)])

    # --- host-side O(N) feature prep ---
    ypos = ((np.arange(N) // W_) % H_).astype(np.int64)
    oneh = np.zeros((N, 16), np.float32); oneh[np.arange(N), ypos] = 1.0
    m16 = (np.abs(np.arange(16)[:, None] - np.arange(16)[None, :]) <= 2).astype(np.float32)
    lfeat = np.concatenate(
        [(2.0 * wood)[None], (2.0 * leaves)[None], 1.5 * wood[None] * oneh.T]
    ).astype(np.float32)                                   # [18, N]
    wood_k = wood[idx_pad]; leaves_k = leaves[idx_pad]
    mk = m16 @ oneh[idx_pad].T                             # [16, nk_pad]
    rfeat = np.concatenate(
        [wood_k[None], leaves_k[None], wood_k[None] * mk]
    ).astype(np.float32)                                   # [18, nk_pad]
    abias = np.zeros(nk_pad, np.float32); abias[nk:] = NEG
    abias = np.ascontiguousarray(abias.reshape(nkc, 128).T)  # [128, nkc]
    xc = np.ascontiguousarray(x2[:, idx_pad])

    gsel = np.zeros((128, 4), np.float32)
    gsel[np.arange(128), np.arange(128) // GSIZE] = 1.0
    ident33 = np.eye(33, dtype=np.float32)

    scale = HD ** -0.5
    shared = {
        "x2": x2, "xc": xc, "lfeat": lfeat, "rfeat": rfeat, "abias": abias,
        "gseli": gsel, "gselt": np.ascontiguousarray(gsel.T), "ident": ident33,
        "gnw": gn_w.reshape(C, 1), "gnb": gn_b.reshape(C, 1),
        "one32": np.ones((1, HD), np.float32),
        "onesk": np.ones((1, nk_pad), np.float32),
        "one32": np.ones((1, HD), np.float32),
        "onesk": np.ones((1, nk_pad), np.float32),
    }
    in_maps = []
    for hd_i in range(NCORES):
        r0 = hd_i * HD
        m = dict(shared)
        m["wq"] = np.ascontiguousarray((qkv_w[0 * C + r0 : 0 * C + r0 + HD] * scale).T)
        m["wk"] = np.ascontiguousarray(qkv_w[1 * C + r0 : 1 * C + r0 + HD].T)
        m["wv"] = np.ascontiguousarray(qkv_w[2 * C + r0 : 2 * C + r0 + HD].T)
        m["bq"] = np.ascontiguousarray((qkv_b[0 * C + r0 : 0 * C + r0 + HD] * scale)[:, None])
        m["bk"] = np.ascontiguousarray(qkv_b[1 * C + r0 : 1 * C + r0 + HD][:, None])
        m["bv"] = np.ascontiguousarray(qkv_b[2 * C + r0 : 2 * C + r0 + HD][:, None])
        m["pwt"] = np.ascontiguousarray(proj_w[:, r0 : r0 + HD].T)
        in_maps.append(m)

    if nk_pad not in _CACHE:
        _CACHE[nk_pad] = _build(nk_pad)
    nc = _CACHE[nk_pad]

    use_trace = TRACE
    if use_trace:
        import importlib.util
        if importlib.util.find_spec("antenv.axon_hooks") is None:
            use_trace = False
    res = run_bass_kernel_spmd(nc, in_maps, core_ids=list(range(NCORES)), trace=use_trace)
    LAST_RESULT["res"] = res

    acc = np.zeros((C, N), np.float32)
    for i in range(NCORES):
        acc += res.results[i]["o"]
    y = x2 + acc + proj_b[:, None]
    return y.reshape(B, C, D_, H_, W_).astype(np.float32)



# revision 30
# speedup vs baseline: 1.4639x; 1.4639x over previous
"""Trainium2 Bass kernel for MinecraftAwareAttention3D (v2).

Full (unsharded) inputs in, full output out. One attention head per NeuronCore
(tensor parallel over num_heads=8); GroupNorm + QKV replicated per core;
per-head softmax numerator/denominator returned to the host, which applies the
normalize + output projection + residual (cheap 4096x256x256 sgemm).

Key structure (chosen against the TimelineSim cost model):
  * All activations bf16: halves input DMA, enables 1-cycle/row PE matmuls
    at any output width and DVE 2x/4x modes for the normalize.
  * Boosts folded into QK^T as 18 extra bf16 contraction rows; air keys
    compacted away on the host (nk_pad ~3072 instead of 4096).
  * exp of the score matrix is split between ACT (hardware Exp, bf16 out)
    and DVE (Schraudolph bit-trick: t = s*S15 + MAGIC; bits<<8 = exp bits;
    f32->bf16 narrowing copy on the idle GpSimd/Pool engine).
  * Transposed PV: out[128q, 33] += pt[128k,128q]^T @ [v|1][128k, 33] per
    key chunk -- full 128-partition output, 33-row bf16 matmuls, and the
    softmax denominator rides along as column 32.
  * V^T built directly by transposed-V matmuls (contraction over channels),
    with the v-bias added via a rank-1 ones-row matmul.
  * No normalize / projection on device: the host divides by the
    denominator and does the 256x256 projection fused with the residual.
"""

import numpy as np

import concourse.bass as bass
import concourse.tile as tile
from concourse import mybir
from concourse.bass_utils import run_bass_kernel_spmd

F32 = mybir.dt.float32
F32R = mybir.dt.float32r
BF16 = mybir.dt.bfloat16
I32 = mybir.dt.int32
AF = mybir.ActivationFunctionType
ALU = mybir.AluOpType

B, C, D_, H_, W_ = 1, 256, 16, 16, 16
N = D_ * H_ * W_          # 4096 spatial positions
HEADS, HD = 8, 32
GROUPS = 8
GSIZE = C // GROUPS
EPS = 1e-5
NEG = -1e9
NF = HD + 18              # fused contraction depth: 32 qk dims + 18 boost dims
NCORES = 8
NQG = N // 1024           # 4 query groups of 1024

# Schraudolph fast-exp constants (2^15 scaling, magic 2^23)
S15 = float((1 << 15) * 1.4426950408889634)
BMAGIC = float((127.0 - 0.0437) * (1 << 15) + (1 << 23))
DVE_PAD_BIAS = -60.0      # pad-key logit bias on DVE chunks (exp ~ e^-60)
DVE_FRAC_NUM, DVE_FRAC_DEN = 7, 24   # ~7/24 of key chunks take the DVE path

TRACE = False             # test.py can flip this for profiling
LAST_RESULT = {}

_CACHE = {}


def _dve_chunk(kc, nkc, qg=1):
    """Evenly spread DVE-assigned key chunks among ACT ones, keeping the
    first chunks and the last three on ACT (ramp/tail latency). In the
    first q-group DVE is still draining phase-1/2 normalizes, so its DVE
    chunks start later."""
    n_dve = (nkc * DVE_FRAC_NUM) // DVE_FRAC_DEN
    lo = 2 if qg == 0 else 1
    if kc < lo or kc >= nkc - 3:
        return False
    j, m = kc - lo, nkc - 3 - lo
    return ((j + 1) * n_dve) // m > (j * n_dve) // m


def _split_waits(nc, max_waits=1):
    """This walrus build only encodes one sync wait per instruction; hoist
    extra waits onto same-engine NOPs inserted just before the instruction."""
    n = 0
    for f in nc.m.functions:
        for bb in f.blocks:
            new_insts = []
            for inst in bb.instructions:
                si = inst.sync_info
                if si is not None and si.on_wait and len(si.on_wait) > max_waits:
                    waits = list(si.on_wait)
                    si.on_wait = waits[-max_waits:]
                    for i in range(0, len(waits) - max_waits, max_waits):
                        n += 1
                        nop = mybir.InstNoOp(name=f"I-wsplit-{n}", ins=[], outs=[])
                        nop.engine = inst.engine
                        nop.sync_info = mybir.SyncInfo(
                            on_wait=waits[i : i + max_waits], on_update=[]
                        )
                        new_insts.append(nop)
                new_insts.append(inst)
            bb.instructions[:] = new_insts
    return n


def _build(nk_pad):
    """Build the per-core Bass module; static on the padded compacted key
    count. All data arrives as ExternalInputs."""
    nkc = nk_pad // 128                       # 128-key chunks
    nks = (nk_pad + 1023) // 1024             # 1024-col xc slices per half
    kslices = [(s, min(s + 512, nk_pad)) for s in range(0, nk_pad, 512)]

    # f32 const-blob column layout: [a_c0, a_c1, b_c0, b_c1, bq, bk | abias | mab]
    A0 = 6                    # abias cols
    M0 = A0 + nkc             # mab cols
    CB32 = M0 + nkc
    # bf16 const-blob column layout
    BV0 = 192                 # bvT row (row 0)
    ON0 = 224                 # ones row (row 0)
    CB16 = ON0 + 128

    nc = bass.Bass()

    # ---- I/O ----
    x2 = nc.dram_tensor("x2", [C, N], BF16, kind="ExternalInput")
    xc = nc.dram_tensor("xc", [C, nk_pad], BF16, kind="ExternalInput")
    lfeat = nc.dram_tensor("lfeat", [18, N], BF16, kind="ExternalInput")
    rfeat = nc.dram_tensor("rfeat", [18, nk_pad], BF16, kind="ExternalInput")
    cb32 = nc.dram_tensor("cb32", [128, CB32], F32, kind="ExternalInput")
    cb16 = nc.dram_tensor("cb16", [128, CB16], BF16, kind="ExternalInput")
    out = nc.dram_tensor("o", [NQG, 128, 264], F32, kind="ExternalOutput")

    with tile.TileContext(nc) as tc:
        with (
            tc.tile_pool(name="consts", bufs=1) as cp,
            tc.tile_pool(name="live", bufs=1) as lp,
            tc.tile_pool(name="small", bufs=2) as sp,
            tc.tile_pool(name="ptpool", bufs=3) as ptp,
            tc.tile_pool(name="tpool", bufs=2) as tp_,
            tc.tile_pool(name="opool", bufs=2) as op,
            tc.tile_pool(name="ps_qkv", bufs=2, space="PSUM") as ps_qkv,
            tc.tile_pool(name="ps_st", bufs=2, space="PSUM") as ps_st,
            tc.tile_pool(name="ps_std", bufs=1, space="PSUM") as ps_std,
            tc.tile_pool(name="ps_pv", bufs=1, space="PSUM") as ps_pv,
        ):
            # ---- long-lived activations ----
            h = [lp.tile([128, N], BF16, name=f"h{c}") for c in range(2)]
            hk = [lp.tile([128, nk_pad], BF16, name=f"hk{c}") for c in range(2)]
            qf = lp.tile([NF, N], BF16)           # Q' = [q*scale ; L]
            kf = lp.tile([NF, nk_pad], BF16)      # K' = [k ; R]
            vt = lp.tile([128, nkc, HD + 1], BF16)  # per-chunk [v ; 1]^T

            # Warm the ACT exp table-set before anything else touches ACT.
            wz = cp.tile([1, 1], F32)
            nc.vector.memset(wz, 0.0)
            wy = cp.tile([1, 1], F32)
            nc.scalar.activation(out=wy, in_=wz, func=AF.Exp, bias=0.0, scale=1.0)
            # zero PE weights: opens each q-group's PSUM accumulation region
            # with a single spanning matmul (a region-sliced start=True
            # clobbers sibling regions in the same PSUM bank on hardware)
            zw = cp.tile([128, 128], BF16)
            nc.vector.memset(zw, 0.0)

            # ================= Phase 1: loads + GroupNorm =================
            # GroupNorm statistics are computed on the host (pure function of
            # the input); the device only applies h = a*x + b. DMA order puts
            # the weights and the first x2/xc slices first so the first QK
            # chunk is ready ~10us in.
            with tc.tile_pool(name="xpool", bufs=1) as xp:
                cb16_t = cp.tile([128, CB16], BF16)
                nc.sync.dma_start(out=cb16_t, in_=cb16[:, :])
                cb32_t = cp.tile([128, CB32], F32)
                nc.sync.dma_start(out=cb32_t, in_=cb32[:, :])
                xt = [xp.tile([128, N], BF16, name=f"xt{c}") for c in range(2)]
                for c in range(2):
                    nc.sync.dma_start(
                        out=xt[c][:, 0:2048],
                        in_=x2[c * 128 : (c + 1) * 128, 0:2048],
                    )
                xcs = []
                for s in range(nks):
                    s0, s1 = s * 1024, min((s + 1) * 1024, nk_pad)
                    pair = []
                    for c in range(2):
                        xs_t = xp.tile(
                            [128, 1024], BF16, name="xcs", tag="xcs", bufs=2 * nks
                        )
                        pair.append(xs_t)
                    xcs.append(pair)
                for c in range(2):
                    nc.sync.dma_start(
                        out=xcs[0][c][:, :], in_=xc[c * 128 : (c + 1) * 128, 0:1024]
                    )
                nc.sync.dma_start(out=qf[HD:NF, :], in_=lfeat[:, :])
                nc.sync.dma_start(out=kf[HD:NF, :], in_=rfeat[:, :])
                for c in range(2):
                    nc.sync.dma_start(
                        out=xt[c][:, 2048:N],
                        in_=x2[c * 128 : (c + 1) * 128, 2048:N],
                    )
                for s in range(1, nks):
                    s0, s1 = s * 1024, min((s + 1) * 1024, nk_pad)
                    for c in range(2):
                        nc.sync.dma_start(
                            out=xcs[s][c][:, 0 : s1 - s0],
                            in_=xc[c * 128 : (c + 1) * 128, s0:s1],
                        )

                # ones column of V'T
                nc.gpsimd.memset(vt[:, :, HD : HD + 1], 1.0)

                ab = [(cb32_t[:, c : c + 1], cb32_t[:, 2 + c : 3 + c]) for c in range(2)]

                # queries: first 512 columns first (unblocks the first QK),
                # then the rest of the first half; key slice 0. Later hk
                # slices and the h second half are emitted just-in-time in
                # the qg0 loop to keep the DVE FIFO unblocked.
                for c in range(2):
                    a_ch, b_ch = ab[c]
                    nc.vector.tensor_scalar(
                        out=h[c][:, 0:512], in0=xt[c][:, 0:512],
                        scalar1=a_ch, scalar2=b_ch, op0=ALU.mult, op1=ALU.add,
                    )
                for c in range(2):
                    a_ch, b_ch = ab[c]
                    nc.vector.tensor_scalar(
                        out=h[c][:, 512:2048], in0=xt[c][:, 512:2048],
                        scalar1=a_ch, scalar2=b_ch, op0=ALU.mult, op1=ALU.add,
                    )

                emitted_hk = set()
                emitted_hrest = [False]

                def emit_hk(s):
                    if s in emitted_hk or s >= nks:
                        return
                    emitted_hk.add(s)
                    s0, s1 = s * 1024, min((s + 1) * 1024, nk_pad)
                    for c in range(2):
                        a_ch, b_ch = ab[c]
                        nc.vector.tensor_scalar(
                            out=hk[c][:, s0:s1], in0=xcs[s][c][:, 0 : s1 - s0],
                            scalar1=a_ch, scalar2=b_ch, op0=ALU.mult, op1=ALU.add,
                        )

                def emit_hrest():
                    if emitted_hrest[0]:
                        return
                    emitted_hrest[0] = True
                    for c in range(2):
                        a_ch, b_ch = ab[c]
                        nc.vector.tensor_scalar(
                            out=h[c][:, 2048:N], in0=xt[c][:, 2048:N],
                            scalar1=a_ch, scalar2=b_ch, op0=ALU.mult, op1=ALU.add,
                        )

                emit_hk(0)

            # ========== Phase 2+3: QKV emission fused into attention ==========
            # Phase-2 work (K slices, Q slices, V^T chunks) is emitted
            # just-in-time inside the first q-group's chunk loop so the PE
            # FIFO never parks early QK matmuls behind V^T chunks that wait
            # on late xc DMA slices.
            emitted_k = set()
            emitted_q = set()
            emitted_vt = set()

            evac_rr = [0]

            def _evacuate(dst_ap, ps_ap, bias_ap):
                # Round-robin the PSUM->SBUF bias-evacuation across ACT/DVE/
                # Pool so consecutive QKV slices pipeline instead of
                # serializing behind one engine's FIFO.
                e = evac_rr[0] % 2
                evac_rr[0] += 1
                if e == 0:
                    nc.scalar.add(out=dst_ap, in_=ps_ap, add=bias_ap)
                else:
                    nc.vector.tensor_scalar_add(out=dst_ap, in0=ps_ap, scalar1=bias_ap)

            def emit_k(j, act=False):
                if j in emitted_k or j >= len(kslices):
                    return
                emitted_k.add(j)
                s0, s1 = kslices[j]
                ps = ps_qkv.tile([128, 512], F32, space="PSUM", name="qkv_ps", tag="s")
                for c in range(2):
                    nc.tensor.matmul(
                        ps[0:HD, 0 : s1 - s0],
                        lhsT=cb16_t[:, 64 + c * HD : 64 + (c + 1) * HD],
                        rhs=hk[c][:, s0:s1],
                        start=(c == 0),
                        stop=(c == 1),
                    )
                _evacuate(kf[0:HD, s0:s1], ps[0:HD, 0 : s1 - s0], cb32_t[0:HD, 5:6])

            def emit_q(i, act=False):
                if i in emitted_q or i >= 8:
                    return
                emitted_q.add(i)
                q0 = i * 512
                ps = ps_qkv.tile([128, 512], F32, space="PSUM", name="qkv_ps", tag="s")
                for c in range(2):
                    nc.tensor.matmul(
                        ps[0:HD, :],
                        lhsT=cb16_t[:, c * HD : (c + 1) * HD],
                        rhs=h[c][:, q0 : q0 + 512],
                        start=(c == 0),
                        stop=(c == 1),
                    )
                _evacuate(qf[0:HD, q0 : q0 + 512], ps[0:HD, :], cb32_t[0:HD, 4:5])

            def emit_vt(kc):
                if kc in emitted_vt or kc >= nkc:
                    return
                emitted_vt.add(kc)
                k0 = kc * 128
                tps = ps_qkv.tile([128, 512], F32, space="PSUM", name="qkv_ps", tag="s")
                for c in range(2):
                    nc.tensor.matmul(
                        tps[:, 0:HD],
                        lhsT=hk[c][:, k0 : k0 + 128],
                        rhs=cb16_t[:, 128 + c * HD : 128 + (c + 1) * HD],
                        start=(c == 0),
                        stop=False,
                    )
                nc.tensor.matmul(
                    tps[:, 0:HD],
                    lhsT=cb16_t[0:1, ON0 : ON0 + 128],
                    rhs=cb16_t[0:1, BV0 : BV0 + HD],
                    start=False,
                    stop=True,
                )
                if kc % 2 == 0:
                    nc.scalar.activation(
                        out=vt[:, kc, 0:HD], in_=tps[:, 0:HD], func=AF.Copy,
                    )
                else:
                    nc.vector.tensor_copy(out=vt[:, kc, 0:HD], in_=tps[:, 0:HD])

            emit_k(0, act=True)
            emit_q(0, act=True)
            emit_q(1, act=True)

            def emit_pvt(kc, pvq, pt, vlhs):
                for qb in range(8):
                    nc.tensor.matmul(
                        pvq[:, qb * (HD + 1) : (qb + 1) * (HD + 1)],
                        lhsT=pt[:, qb * 128 : (qb + 1) * 128],
                        rhs=vlhs,
                        start=False,
                        stop=(kc == nkc - 1),
                        skip_group_check=True,
                    )

            if True:
                for qg in range(NQG):
                    q0 = qg * 1024
                    pvq = ps_pv.tile([128, 8 * (HD + 1)], F32, space="PSUM", name="pvq")
                    nc.tensor.matmul(
                        pvq, lhsT=zw, rhs=cb16_t[:, 0 : 8 * (HD + 1)],
                        start=True, stop=False, skip_group_check=True,
                    )

                    sts = {}

                    def issue_qk(kc):
                        emit_k(kc // 4)
                        lhs = kf[:, kc * 128 : (kc + 1) * 128]
                        if _dve_chunk(kc, nkc, qg):
                            sta = ps_std.tile([128, 512], F32, space="PSUM", name="std")
                            stb = ps_std.tile([128, 512], F32, space="PSUM", name="std")
                            nc.tensor.matmul(
                                sta, lhsT=lhs, rhs=qf[:, q0 : q0 + 512],
                                start=True, stop=True,
                            )
                            nc.tensor.matmul(
                                stb, lhsT=lhs, rhs=qf[:, q0 + 512 : q0 + 1024],
                                start=True, stop=True,
                            )
                            sts[kc] = (sta, stb)
                        else:
                            st = ps_st.tile([128, 1024], F32, space="PSUM", name="st")
                            nc.tensor.matmul(
                                st[:, 0:512], lhsT=lhs, rhs=qf[:, q0 : q0 + 512],
                                start=True, stop=True,
                            )
                            nc.tensor.matmul(
                                st[:, 512:1024], lhsT=lhs,
                                rhs=qf[:, q0 + 512 : q0 + 1024],
                                start=True, stop=True,
                            )
                            sts[kc] = st

                    issue_qk(0)
                    issue_qk(1)
                    if qg == 0:
                        emit_vt(0)
                        emit_vt(1)
                    deferred_pvt = []
                    for kc in range(nkc):
                        st = sts.pop(kc)
                        if deferred_pvt and kc >= 3:
                            # flush before this chunk's pt allocation reuses
                            # a pool slot still referenced by a deferred PVT
                            for a in deferred_pvt:
                                emit_pvt(*a)
                            deferred_pvt = []
                        pt = ptp.tile([128, 1024], BF16, name="pt")
                        if _dve_chunk(kc, nkc, qg):
                            # Schraudolph fast-exp on DVE + narrowing on Pool
                            sta, stb = st
                            t = tp_.tile([128, 1024], F32, name="t")
                            for hf, sth in ((0, sta), (1, stb)):
                                nc.vector.tensor_scalar(
                                    out=t[:, hf * 512 : (hf + 1) * 512], in0=sth,
                                    scalar1=S15,
                                    scalar2=cb32_t[:, M0 + kc : M0 + kc + 1],
                                    op0=ALU.mult, op1=ALU.add,
                                )
                            nc.vector.tensor_scalar(
                                out=t.bitcast(I32), in0=t.bitcast(I32),
                                scalar1=8, scalar2=None,
                                op0=ALU.logical_shift_left,
                            )
                            nc.gpsimd.tensor_copy(out=pt, in_=t)
                        else:
                            nc.scalar.activation(
                                out=pt, in_=st, func=AF.Exp,
                                bias=cb32_t[:, A0 + kc : A0 + kc + 1], scale=1.0,
                            )
                        if qg == 0:
                            emit_vt(kc + 2)
                            if kc == 2:
                                emit_hk(1)
                            if kc == 6:
                                emit_hk(2)
                                emit_hrest()
                            if kc >= 3:
                                emit_q(2 + (kc - 3) // 3)
                        if kc + 2 < nkc:
                            issue_qk(kc + 2)
                        pvt_args = (kc, pvq, pt, vt[:, kc, :])
                        if qg > 0 and kc < 3:
                            deferred_pvt.append(pvt_args)
                            continue
                        emit_pvt(*pvt_args)
                    for j in range(len(kslices)):
                        emit_k(j)
                    for i in range(8):
                        emit_q(i)
                    for kc2 in range(nkc):
                        emit_vt(kc2)
                    hp = tc.high_priority()
                    hp.__enter__()
                    ot = op.tile([128, 8 * (HD + 1)], F32, name="ot")
                    nc.vector.tensor_copy(out=ot, in_=pvq)
                    nc.sync.dma_start(out=out[qg, :, :], in_=ot)
                    hp.__exit__(None, None, None)

    _split_waits(nc)
    return nc


def _numpy_reference(x, block_types, gn_w, gn_b, qkv_w, qkv_b, proj_w, proj_b,
                     is_air, is_wood, is_leaves):
    """Pure-numpy fallback (degenerate case: no non-air keys)."""
    xf = x.reshape(B, C, N).astype(np.float64)
    xs = xf.reshape(B, GROUPS, GSIZE * N)
    mu = xs.mean(axis=2, keepdims=True)
    var = xs.var(axis=2, keepdims=True)
    hh = ((xs - mu) / np.sqrt(var + EPS)).reshape(B, C, N)
    hh = hh * gn_w[None, :, None] + gn_b[None, :, None]
    qkv = np.einsum("oc,bcn->bon", qkv_w.astype(np.float64), hh) + qkv_b[None, :, None]
    qkv = qkv.reshape(B, 3, HEADS, HD, N)
    q, k, v = qkv[:, 0], qkv[:, 1], qkv[:, 2]
    attn = np.einsum("bhdn,bhdm->bhnm", q, k) * (HD ** -0.5)
    bf = block_types.reshape(B, N)
    air = is_air[bf]; wood = is_wood[bf]; leaves = is_leaves[bf]
    attn = np.where(air[:, None, None, :] > 0, NEG, attn)
    wo = wood[:, :, None] * wood[:, None, :]
    lo = leaves[:, :, None] * leaves[:, None, :]
    mb = np.clip((wo + lo) * 2.0, 0.0, 10.0)
    pos = np.arange(N); ypos = (pos // W_) % H_
    vm = (np.abs(ypos[None, :] - ypos[:, None]) <= 2).astype(np.float64)
    vb = np.clip(wo * vm[None] * 1.5, 0.0, 10.0)
    attn = attn + (mb + vb)[:, None]
    attn = attn - attn.max(axis=-1, keepdims=True)
    e = np.exp(attn); p = e / e.sum(axis=-1, keepdims=True)
    o = np.einsum("bhnm,bhdm->bhdn", p, v).reshape(B, C, N)
    o = np.einsum("oc,bcn->bon", proj_w.astype(np.float64), o) + proj_b[None, :, None]
    return (xf + o).reshape(x.shape).astype(np.float32)


def kernel(x, block_types, gn_w, gn_b, qkv_w, qkv_b, proj_w, proj_b,
           is_air, is_wood, is_leaves):
    import ml_dtypes
    BF = ml_dtypes.bfloat16

    x = np.ascontiguousarray(np.asarray(x, dtype=np.float32))
    gn_w = np.asarray(gn_w, np.float32); gn_b = np.asarray(gn_b, np.float32)
    qkv_w = np.asarray(qkv_w, np.float32); qkv_b = np.asarray(qkv_b, np.float32)
    proj_w = np.asarray(proj_w, np.float32); proj_b = np.asarray(proj_b, np.float32)
    is_air = np.asarray(is_air, np.float32)
    is_wood = np.asarray(is_wood, np.float32)
    is_leaves = np.asarray(is_leaves, np.float32)
    bt = np.asarray(block_types).reshape(N).astype(np.int64)

    x2 = x.reshape(C, N)
    air = is_air[bt]; wood = is_wood[bt]; leaves = is_leaves[bt]
    idx = np.nonzero(air <= 0.0)[0]
    nk = len(idx)
    if nk == 0:
        return _numpy_reference(x, block_types, gn_w, gn_b, qkv_w, qkv_b,
                                proj_w, proj_b, is_air, is_wood, is_leaves)

    nk_pad = ((nk + 127) // 128) * 128
    nkc = nk_pad // 128
    idx_pad = np.concatenate([idx, np.full(nk_pad - nk, idx[0], np.int64)])

    # --- host-side O(N) feature prep ---
    ypos = ((np.arange(N) // W_) % H_).astype(np.int64)
    oneh = np.zeros((N, 16), np.float32); oneh[np.arange(N), ypos] = 1.0
    m16 = (np.abs(np.arange(16)[:, None] - np.arange(16)[None, :]) <= 2).astype(np.float32)
    lfeat = np.concatenate(
        [(2.0 * wood)[None], (2.0 * leaves)[None], 1.5 * wood[None] * oneh.T]
    ).astype(BF)                                            # [18, N]
    wood_k = wood[idx_pad]; leaves_k = leaves[idx_pad]
    mk = m16 @ oneh[idx_pad].T                              # [16, nk_pad]
    rfeat = np.concatenate(
        [wood_k[None], leaves_k[None], wood_k[None] * mk]
    ).astype(BF)                                            # [18, nk_pad]

    pad_col = np.zeros(nk_pad, np.float32); pad_col[nk:] = 1.0
    pad_m = np.ascontiguousarray(pad_col.reshape(nkc, 128).T)  # [128, nkc]
    abias = pad_m * NEG
    mab = BMAGIC + (pad_m * DVE_PAD_BIAS) * S15

    # GroupNorm statistics on the host (f32, matches the reference exactly)
    xg = x2.reshape(GROUPS, GSIZE * N)
    mu_g = xg.mean(axis=1)
    var_g = xg.var(axis=1)
    rstd_g = 1.0 / np.sqrt(var_g + EPS)
    mu_ch = np.repeat(mu_g, GSIZE); rstd_ch = np.repeat(rstd_g, GSIZE)
    a_ch = (gn_w * rstd_ch).astype(np.float32)
    b_ch = (gn_b - mu_ch * a_ch).astype(np.float32)

    # f32 const blob: [a_c0, a_c1, b_c0, b_c1, bq, bk | abias | mab]
    A0 = 6; M0 = A0 + nkc; CB32 = M0 + nkc
    scale = HD ** -0.5
    cb32_shared = np.zeros((128, CB32), np.float32)
    cb32_shared[:, 0] = a_ch[0:128]; cb32_shared[:, 1] = a_ch[128:256]
    cb32_shared[:, 2] = b_ch[0:128]; cb32_shared[:, 3] = b_ch[128:256]
    cb32_shared[:, A0:M0] = abias
    cb32_shared[:, M0:CB32] = mab

    BV0 = 192; ON0 = 224; CB16 = ON0 + 128

    x2b = np.ascontiguousarray(x2.astype(BF))
    xcb = np.ascontiguousarray(x2[:, idx_pad].astype(BF))

    shared = {
        "x2": x2b, "xc": xcb, "lfeat": np.ascontiguousarray(lfeat),
        "rfeat": np.ascontiguousarray(rfeat),
    }
    in_maps = []
    for hd_i in range(NCORES):
        r0 = hd_i * HD
        cb32_i = cb32_shared.copy()
        cb32_i[0:HD, 4] = qkv_b[0 * C + r0 : 0 * C + r0 + HD] * scale
        cb32_i[0:HD, 5] = qkv_b[1 * C + r0 : 1 * C + r0 + HD]
        cb16_i = np.zeros((128, CB16), np.float32)
        cb16_i[:, 0:HD] = qkv_w[0 * C + r0 : 0 * C + r0 + HD, 0:128].T * scale
        cb16_i[:, HD:2 * HD] = qkv_w[0 * C + r0 : 0 * C + r0 + HD, 128:256].T * scale
        cb16_i[:, 64:64 + HD] = qkv_w[1 * C + r0 : 1 * C + r0 + HD, 0:128].T
        cb16_i[:, 64 + HD:128] = qkv_w[1 * C + r0 : 1 * C + r0 + HD, 128:256].T
        cb16_i[:, 128:128 + HD] = qkv_w[2 * C + r0 : 2 * C + r0 + HD, 0:128].T
        cb16_i[:, 128 + HD:192] = qkv_w[2 * C + r0 : 2 * C + r0 + HD, 128:256].T
        cb16_i[0, BV0:BV0 + HD] = qkv_b[2 * C + r0 : 2 * C + r0 + HD]
        cb16_i[0, ON0:CB16] = 1.0
        m = dict(shared)
        m["cb32"] = np.ascontiguousarray(cb32_i)
        m["cb16"] = np.ascontiguousarray(cb16_i.astype(BF))
        in_maps.append(m)

    if nk_pad not in _CACHE:
        _CACHE[nk_pad] = _build(nk_pad)
    nc = _CACHE[nk_pad]

    use_trace = TRACE
    if use_trace:
        import importlib.util
        if importlib.util.find_spec("antenv.axon_hooks") is None:
            use_trace = False
    res = run_bass_kernel_spmd(nc, in_maps, core_ids=list(range(NCORES)), trace=use_trace)
    LAST_RESULT["res"] = res

    # host: normalize + projection + residual
    attn_all = np.empty((N, C), np.float32)
    for i in range(NCORES):
        o = np.asarray(res.results[i]["o"], np.float32)        # [4, 128, 264]
        oh = o.reshape(NQG, 128, 8, HD + 1).transpose(0, 2, 1, 3).reshape(N, HD + 1)
        attn_all[:, i * HD : (i + 1) * HD] = oh[:, 0:HD] / oh[:, HD : HD + 1]
    y = x2 + proj_w @ attn_all.T.astype(np.float32) + proj_b[:, None]
    return y.reshape(B, C, D_, H_, W_).astype(np.float32)


# revision 49
# speedup vs baseline: 1.5689x; 1.0717x over previous
"""Trainium2 Bass kernel for MinecraftAwareAttention3D (v2).

Full (unsharded) inputs in, full output out. One attention head per NeuronCore
(tensor parallel over num_heads=8); GroupNorm + QKV replicated per core;
per-head softmax numerator/denominator returned to the host, which applies the
normalize + output projection + residual (cheap 4096x256x256 sgemm).

Key structure (chosen against the TimelineSim cost model):
  * All activations bf16: halves input DMA, enables 1-cycle/row PE matmuls
    at any output width and DVE 2x/4x modes for the normalize.
  * Boosts folded into QK^T as 18 extra bf16 contraction rows; air keys
    compacted away on the host (nk_pad ~3072 instead of 4096).
  * exp of the score matrix is split between ACT (hardware Exp, bf16 out)
    and DVE (Schraudolph bit-trick: t = s*S15 + MAGIC; bits<<8 = exp bits;
    f32->bf16 narrowing copy on the idle GpSimd/Pool engine).
  * Transposed PV: out[128q, 33] += pt[128k,128q]^T @ [v|1][128k, 33] per
    key chunk -- full 128-partition output, 33-row bf16 matmuls, and the
    softmax denominator rides along as column 32.
  * V^T built directly by transposed-V matmuls (contraction over channels),
    with the v-bias added via a rank-1 ones-row matmul.
  * No normalize / projection on device: the host divides by the
    denominator and does the 256x256 projection fused with the residual.
"""

import numpy as np

import concourse.bass as bass
import concourse.tile as tile
from concourse import mybir
from concourse.bass_utils import run_bass_kernel_spmd

F32 = mybir.dt.float32
F32R = mybir.dt.float32r
BF16 = mybir.dt.bfloat16
I32 = mybir.dt.int32
AF = mybir.ActivationFunctionType
ALU = mybir.AluOpType

B, C, D_, H_, W_ = 1, 256, 16, 16, 16
N = D_ * H_ * W_          # 4096 spatial positions
HEADS, HD = 8, 32
GROUPS = 8
GSIZE = C // GROUPS
EPS = 1e-5
NEG = -1e9
NF = HD + 18              # fused contraction depth: 32 qk dims + 18 boost dims
NCORES = 8
NQG = N // 1024           # 4 query groups of 1024

# Schraudolph fast-exp constants (2^15 scaling, magic 2^23)
S15 = float((1 << 15) * 1.4426950408889634)
BMAGIC = float((127.0 - 0.0437) * (1 << 15) + (1 << 23))
DVE_PAD_BIAS = -60.0      # pad-key logit bias on DVE chunks (exp ~ e^-60)
DVE_FRAC_NUM, DVE_FRAC_DEN = 7, 24   # ~7/24 of key chunks take the DVE path

TRACE = False             # test.py can flip this for profiling
LAST_RESULT = {}

_CACHE = {}


def _dve_chunk(kc, nkc, qg=1):
    """Evenly spread DVE-assigned key chunks among ACT ones, keeping the
    first chunks and the last three on ACT (ramp/tail latency). In the
    first q-group DVE is still draining phase-1/2 normalizes, so its DVE
    chunks start later."""
    n_dve = (nkc * DVE_FRAC_NUM) // DVE_FRAC_DEN
    if qg == 0:
        n_dve = (nkc * 5) // 24
    lo = 2 if qg == 0 else 1
    if kc < lo or kc >= nkc - 5:
        return False
    j, m = kc - lo, nkc - 3 - lo
    return ((j + 1) * n_dve) // m > (j * n_dve) // m


def _split_waits(nc, max_waits=1):
    """This walrus build only encodes one sync wait per instruction; hoist
    extra waits onto same-engine NOPs inserted just before the instruction."""
    n = 0
    for f in nc.m.functions:
        for bb in f.blocks:
            new_insts = []
            for inst in bb.instructions:
                si = inst.sync_info
                if si is not None and si.on_wait and len(si.on_wait) > max_waits:
                    waits = list(si.on_wait)
                    si.on_wait = waits[-max_waits:]
                    for i in range(0, len(waits) - max_waits, max_waits):
                        n += 1
                        nop = mybir.InstNoOp(name=f"I-wsplit-{n}", ins=[], outs=[])
                        nop.engine = inst.engine
                        nop.sync_info = mybir.SyncInfo(
                            on_wait=waits[i : i + max_waits], on_update=[]
                        )
                        new_insts.append(nop)
                new_insts.append(inst)
            bb.instructions[:] = new_insts
    return n


def _build(nk_pad):
    """Build the per-core Bass module; static on the padded compacted key
    count. All data arrives as ExternalInputs."""
    nkc = nk_pad // 128                       # 128-key chunks
    nks = (nk_pad + 1023) // 1024             # 1024-col xc slices per half
    kslices = [(s, min(s + 512, nk_pad)) for s in range(0, nk_pad, 512)]

    # f32 const-blob column layout: [a_c0, a_c1, b_c0, b_c1, bq, bk | abias | mab]
    A0 = 6                    # abias cols
    M0 = A0 + nkc             # mab cols
    CB32 = M0 + nkc
    # bf16 const-blob column layout
    BV0 = 192                 # bvT row (row 0)
    ON0 = 224                 # ones row (row 0)
    CB16 = ON0 + 128

    nc = bass.Bass()

    # ---- I/O ----
    x2 = nc.dram_tensor("x2", [C, N], BF16, kind="ExternalInput")
    xc = nc.dram_tensor("xc", [C, nk_pad], BF16, kind="ExternalInput")
    lfeat = nc.dram_tensor("lfeat", [18, N], BF16, kind="ExternalInput")
    rfeat = nc.dram_tensor("rfeat", [18, nk_pad], BF16, kind="ExternalInput")
    cb32 = nc.dram_tensor("cb32", [128, CB32], F32, kind="ExternalInput")
    cb16 = nc.dram_tensor("cb16", [128, CB16], BF16, kind="ExternalInput")
    out = nc.dram_tensor("o", [NQG, 128, 264], F32, kind="ExternalOutput")

    with tile.TileContext(nc) as tc:
        with (
            tc.tile_pool(name="consts", bufs=1) as cp,
            tc.tile_pool(name="live", bufs=1) as lp,
            tc.tile_pool(name="small", bufs=2) as sp,
            tc.tile_pool(name="ptpool", bufs=7) as ptp,
            tc.tile_pool(name="tpool", bufs=2) as tp_,
            tc.tile_pool(name="opool", bufs=2) as op,
            tc.tile_pool(name="ps_qkv", bufs=2, space="PSUM") as ps_qkv,
            tc.tile_pool(name="ps_st", bufs=2, space="PSUM") as ps_st,
            tc.tile_pool(name="ps_std", bufs=1, space="PSUM") as ps_std,
            tc.tile_pool(name="ps_pv", bufs=1, space="PSUM") as ps_pv,
        ):
            # ---- long-lived activations ----
            h = [lp.tile([128, N], BF16, name=f"h{c}") for c in range(2)]
            hk = [lp.tile([128, nk_pad], BF16, name=f"hk{c}") for c in range(2)]
            qf = lp.tile([NF, N], BF16)           # Q' = [q*scale ; L]
            kf = lp.tile([NF, nk_pad], BF16)      # K' = [k ; R]
            vt = lp.tile([128, nkc, HD + 1], BF16)  # per-chunk [v ; 1]^T

            # Warm the ACT exp table-set before anything else touches ACT.
            wz = cp.tile([1, 1], F32)
            nc.vector.memset(wz, 0.0)
            wy = cp.tile([1, 1], F32)
            nc.scalar.activation(out=wy, in_=wz, func=AF.Exp, bias=0.0, scale=1.0)
            # zero PE weights: opens each q-group's PSUM accumulation region
            # with a single spanning matmul (a region-sliced start=True
            # clobbers sibling regions in the same PSUM bank on hardware)
            zw = cp.tile([128, 128], BF16)
            nc.vector.memset(zw, 0.0)

            # ================= Phase 1: loads + GroupNorm =================
            # GroupNorm statistics are computed on the host (pure function of
            # the input); the device only applies h = a*x + b. DMA order puts
            # the weights and the first x2/xc slices first so the first QK
            # chunk is ready ~10us in.
            with tc.tile_pool(name="xpool", bufs=1) as xp:
                cb16_t = cp.tile([128, CB16], BF16)
                nc.sync.dma_start(out=cb16_t, in_=cb16[:, :])
                cb32_t = cp.tile([128, CB32], F32)
                nc.sync.dma_start(out=cb32_t, in_=cb32[:, :])
                xt = [xp.tile([128, N], BF16, name=f"xt{c}") for c in range(2)]
                for c in range(2):
                    nc.sync.dma_start(
                        out=xt[c][:, 0:512],
                        in_=x2[c * 128 : (c + 1) * 128, 0:512],
                    )
                xcs = []
                for s in range(nks):
                    s0, s1 = s * 1024, min((s + 1) * 1024, nk_pad)
                    pair = []
                    for c in range(2):
                        xs_t = xp.tile(
                            [128, 1024], BF16, name="xcs", tag="xcs", bufs=2 * nks
                        )
                        pair.append(xs_t)
                    xcs.append(pair)
                for c in range(2):
                    nc.sync.dma_start(
                        out=xcs[0][c][:, 0:512],
                        in_=xc[c * 128 : (c + 1) * 128, 0:512],
                    )
                for c in range(2):
                    nc.sync.dma_start(
                        out=xt[c][:, 512:2048],
                        in_=x2[c * 128 : (c + 1) * 128, 512:2048],
                    )
                for c in range(2):
                    nc.sync.dma_start(
                        out=xcs[0][c][:, 512:1024],
                        in_=xc[c * 128 : (c + 1) * 128, 512:1024],
                    )
                nc.sync.dma_start(out=qf[HD:NF, :], in_=lfeat[:, :])
                nc.sync.dma_start(out=kf[HD:NF, :], in_=rfeat[:, :])
                for c in range(2):
                    nc.sync.dma_start(
                        out=xt[c][:, 2048:N],
                        in_=x2[c * 128 : (c + 1) * 128, 2048:N],
                    )
                for s in range(1, nks):
                    s0, s1 = s * 1024, min((s + 1) * 1024, nk_pad)
                    for c in range(2):
                        nc.sync.dma_start(
                            out=xcs[s][c][:, 0 : s1 - s0],
                            in_=xc[c * 128 : (c + 1) * 128, s0:s1],
                        )

                # ones column of V'T
                nc.gpsimd.memset(vt[:, :, HD : HD + 1], 1.0)

                ab = [(cb32_t[:, c : c + 1], cb32_t[:, 2 + c : 3 + c]) for c in range(2)]

                # queries: first 512 columns first (unblocks the first QK),
                # then the rest of the first half; key slice 0. Later hk
                # slices and the h second half are emitted just-in-time in
                # the qg0 loop to keep the DVE FIFO unblocked.
                for c in range(2):
                    a_ch, b_ch = ab[c]
                    nc.vector.tensor_scalar(
                        out=h[c][:, 0:512], in0=xt[c][:, 0:512],
                        scalar1=a_ch, scalar2=b_ch, op0=ALU.mult, op1=ALU.add,
                    )
                for c in range(2):
                    a_ch, b_ch = ab[c]
                    nc.vector.tensor_scalar(
                        out=h[c][:, 512:2048], in0=xt[c][:, 512:2048],
                        scalar1=a_ch, scalar2=b_ch, op0=ALU.mult, op1=ALU.add,
                    )

                emitted_hk = set()
                emitted_hrest = [False]

                def emit_hk(s, parts=1):
                    if s in emitted_hk or s >= nks:
                        return
                    emitted_hk.add(s)
                    s0, s1 = s * 1024, min((s + 1) * 1024, nk_pad)
                    bounds = [s0 + (s1 - s0) * i // parts for i in range(parts + 1)]
                    for p in range(parts):
                        for c in range(2):
                            a_ch, b_ch = ab[c]
                            nc.vector.tensor_scalar(
                                out=hk[c][:, bounds[p] : bounds[p + 1]],
                                in0=xcs[s][c][:, bounds[p] - s0 : bounds[p + 1] - s0],
                                scalar1=a_ch, scalar2=b_ch,
                                op0=ALU.mult, op1=ALU.add,
                            )

                def emit_hrest():
                    if emitted_hrest[0]:
                        return
                    emitted_hrest[0] = True
                    for c in range(2):
                        a_ch, b_ch = ab[c]
                        nc.vector.tensor_scalar(
                            out=h[c][:, 2048:N], in0=xt[c][:, 2048:N],
                            scalar1=a_ch, scalar2=b_ch, op0=ALU.mult, op1=ALU.add,
                        )

                emit_hk(0, parts=2)

            # ========== Phase 2+3: QKV emission fused into attention ==========
            # Phase-2 work (K slices, Q slices, V^T chunks) is emitted
            # just-in-time inside the first q-group's chunk loop so the PE
            # FIFO never parks early QK matmuls behind V^T chunks that wait
            # on late xc DMA slices.
            emitted_k = set()
            emitted_q = set()
            emitted_vt = set()

            evac_rr = [0]

            def _evacuate(dst_ap, ps_ap, bias_ap):
                # Round-robin the PSUM->SBUF bias-evacuation across ACT/DVE/
                # Pool so consecutive QKV slices pipeline instead of
                # serializing behind one engine's FIFO.
                e = evac_rr[0] % 2
                evac_rr[0] += 1
                if e == 0:
                    nc.scalar.add(out=dst_ap, in_=ps_ap, add=bias_ap)
                else:
                    nc.vector.tensor_scalar_add(out=dst_ap, in0=ps_ap, scalar1=bias_ap)

            def emit_k(j, act=False):
                if j in emitted_k or j >= len(kslices):
                    return
                emitted_k.add(j)
                s0, s1 = kslices[j]
                ps = ps_qkv.tile([128, 512], F32, space="PSUM", name="qkv_ps", tag="s")
                for c in range(2):
                    nc.tensor.matmul(
                        ps[0:HD, 0 : s1 - s0],
                        lhsT=cb16_t[:, 64 + c * HD : 64 + (c + 1) * HD],
                        rhs=hk[c][:, s0:s1],
                        start=(c == 0),
                        stop=(c == 1),
                    )
                _evacuate(kf[0:HD, s0:s1], ps[0:HD, 0 : s1 - s0], cb32_t[0:HD, 5:6])

            def emit_q(i, act=False):
                if i in emitted_q or i >= 8:
                    return
                emitted_q.add(i)
                q0 = i * 512
                ps = ps_qkv.tile([128, 512], F32, space="PSUM", name="qkv_ps", tag="s")
                for c in range(2):
                    nc.tensor.matmul(
                        ps[0:HD, :],
                        lhsT=cb16_t[:, c * HD : (c + 1) * HD],
                        rhs=h[c][:, q0 : q0 + 512],
                        start=(c == 0),
                        stop=(c == 1),
                    )
                _evacuate(qf[0:HD, q0 : q0 + 512], ps[0:HD, :], cb32_t[0:HD, 4:5])

            def emit_vt(kc):
                if kc in emitted_vt or kc >= nkc:
                    return
                emitted_vt.add(kc)
                k0 = kc * 128
                tps = ps_qkv.tile([128, 512], F32, space="PSUM", name="qkv_ps", tag="s")
                for c in range(2):
                    nc.tensor.matmul(
                        tps[:, 0:HD],
                        lhsT=hk[c][:, k0 : k0 + 128],
                        rhs=cb16_t[:, 128 + c * HD : 128 + (c + 1) * HD],
                        start=(c == 0),
                        stop=False,
                    )
                nc.tensor.matmul(
                    tps[:, 0:HD],
                    lhsT=cb16_t[0:1, ON0 : ON0 + 128],
                    rhs=cb16_t[0:1, BV0 : BV0 + HD],
                    start=False,
                    stop=True,
                )
                if kc % 2 == 0:
                    nc.scalar.activation(
                        out=vt[:, kc, 0:HD], in_=tps[:, 0:HD], func=AF.Copy,
                    )
                else:
                    nc.vector.tensor_copy(out=vt[:, kc, 0:HD], in_=tps[:, 0:HD])

            emit_k(0, act=True)
            emit_q(0, act=True)
            emit_q(1, act=True)

            def emit_pvt(kc, pvq, pt, vlhs):
                for qb in range(8):
                    nc.tensor.matmul(
                        pvq[:, qb * (HD + 1) : (qb + 1) * (HD + 1)],
                        lhsT=pt[:, qb * 128 : (qb + 1) * 128],
                        rhs=vlhs,
                        start=False,
                        stop=(kc == nkc - 1),
                        skip_group_check=True,
                    )

            if True:
                total = NQG * nkc
                sts = {}
                pvqs = {}
                pending_pvt = []

                def issue_qk(g):
                    if g >= total:
                        return
                    qg, kc = divmod(g, nkc)
                    q0 = qg * 1024
                    emit_k(kc // 4)
                    lhs = kf[:, kc * 128 : (kc + 1) * 128]
                    if _dve_chunk(kc, nkc, qg):
                        sta = ps_std.tile([128, 512], F32, space="PSUM", name="std")
                        stb = ps_std.tile([128, 512], F32, space="PSUM", name="std")
                        nc.tensor.matmul(
                            sta, lhsT=lhs, rhs=qf[:, q0 : q0 + 512],
                            start=True, stop=True,
                        )
                        nc.tensor.matmul(
                            stb, lhsT=lhs, rhs=qf[:, q0 + 512 : q0 + 1024],
                            start=True, stop=True,
                        )
                        sts[g] = (sta, stb)
                    else:
                        st = ps_st.tile([128, 1024], F32, space="PSUM", name="st")
                        nc.tensor.matmul(
                            st[:, 0:512], lhsT=lhs, rhs=qf[:, q0 : q0 + 512],
                            start=True, stop=True,
                        )
                        nc.tensor.matmul(
                            st[:, 512:1024], lhsT=lhs,
                            rhs=qf[:, q0 + 512 : q0 + 1024],
                            start=True, stop=True,
                        )
                        sts[g] = st

                def get_pvq(qg):
                    if qg not in pvqs:
                        pvq = ps_pv.tile(
                            [128, 8 * (HD + 1)], F32, space="PSUM", name="pvq"
                        )
                        nc.tensor.matmul(
                            pvq, lhsT=zw, rhs=cb16_t[:, 0 : 8 * (HD + 1)],
                            start=True, stop=False, skip_group_check=True,
                        )
                        pvqs[qg] = pvq
                    return pvqs[qg]

                def flush_pvt(up_to=None):
                    while pending_pvt and (up_to is None or pending_pvt[0][0] <= up_to):
                        g2, pt2 = pending_pvt.pop(0)
                        qg2, kc2 = divmod(g2, nkc)
                        emit_pvt(kc2, get_pvq(qg2), pt2, vt[:, kc2, :])
                        if kc2 == nkc - 1:
                            finish_qg(qg2)

                def finish_qg(qg2):
                    hp = tc.high_priority()
                    hp.__enter__()
                    ot = op.tile([128, 8 * (HD + 1)], F32, name="ot")
                    nc.vector.tensor_copy(out=ot, in_=pvqs.pop(qg2))
                    nc.sync.dma_start(out=out[qg2, :, :], in_=ot)
                    hp.__exit__(None, None, None)

                issue_qk(0)
                issue_qk(1)
                for g in range(total):
                    qg, kc = divmod(g, nkc)
                    st = sts.pop(g)
                    pt = ptp.tile([128, 1024], BF16, name="pt")
                    if _dve_chunk(kc, nkc, qg):
                        # Schraudolph fast-exp on DVE + narrowing on Pool
                        sta, stb = st
                        t = tp_.tile([128, 1024], F32, name="t")
                        for hf, sth in ((0, sta), (1, stb)):
                            nc.vector.tensor_scalar(
                                out=t[:, hf * 512 : (hf + 1) * 512], in0=sth,
                                scalar1=S15,
                                scalar2=cb32_t[:, M0 + kc : M0 + kc + 1],
                                op0=ALU.mult, op1=ALU.add,
                            )
                        nc.vector.tensor_scalar(
                            out=t.bitcast(I32), in0=t.bitcast(I32),
                            scalar1=8, scalar2=None,
                            op0=ALU.logical_shift_left,
                        )
                        nc.gpsimd.tensor_copy(out=pt, in_=t)
                    else:
                        nc.scalar.activation(
                            out=pt, in_=st, func=AF.Exp,
                            bias=cb32_t[:, A0 + kc : A0 + kc + 1], scale=1.0,
                        )
                    if qg == 0:
                        emit_vt(kc + 2)
                        if kc == 2:
                            emit_hk(1)
                        if kc == 6:
                            emit_hk(2)
                            emit_hrest()
                        if kc >= 3:
                            emit_q(2 + (kc - 3) // 3)
                    if g == nkc - 3:
                        for j in range(len(kslices)):
                            emit_k(j)
                        for i in range(8):
                            emit_q(i)
                        for kc2 in range(nkc):
                            emit_vt(kc2)
                    issue_qk(g + 2)
                    flush_pvt(up_to=g - 6)
                    pending_pvt.append((g, pt))
                flush_pvt()

    _split_waits(nc)
    return nc


def _numpy_reference(x, block_types, gn_w, gn_b, qkv_w, qkv_b, proj_w, proj_b,
                     is_air, is_wood, is_leaves):
    """Pure-numpy fallback (degenerate case: no non-air keys)."""
    xf = x.reshape(B, C, N).astype(np.float64)
    xs = xf.reshape(B, GROUPS, GSIZE * N)
    mu = xs.mean(axis=2, keepdims=True)
    var = xs.var(axis=2, keepdims=True)
    hh = ((xs - mu) / np.sqrt(var + EPS)).reshape(B, C, N)
    hh = hh * gn_w[None, :, None] + gn_b[None, :, None]
    qkv = np.einsum("oc,bcn->bon", qkv_w.astype(np.float64), hh) + qkv_b[None, :, None]
    qkv = qkv.reshape(B, 3, HEADS, HD, N)
    q, k, v = qkv[:, 0], qkv[:, 1], qkv[:, 2]
    attn = np.einsum("bhdn,bhdm->bhnm", q, k) * (HD ** -0.5)
    bf = block_types.reshape(B, N)
    air = is_air[bf]; wood = is_wood[bf]; leaves = is_leaves[bf]
    attn = np.where(air[:, None, None, :] > 0, NEG, attn)
    wo = wood[:, :, None] * wood[:, None, :]
    lo = leaves[:, :, None] * leaves[:, None, :]
    mb = np.clip((wo + lo) * 2.0, 0.0, 10.0)
    pos = np.arange(N); ypos = (pos // W_) % H_
    vm = (np.abs(ypos[None, :] - ypos[:, None]) <= 2).astype(np.float64)
    vb = np.clip(wo * vm[None] * 1.5, 0.0, 10.0)
    attn = attn + (mb + vb)[:, None]
    attn = attn - attn.max(axis=-1, keepdims=True)
    e = np.exp(attn); p = e / e.sum(axis=-1, keepdims=True)
    o = np.einsum("bhnm,bhdm->bhdn", p, v).reshape(B, C, N)
    o = np.einsum("oc,bcn->bon", proj_w.astype(np.float64), o) + proj_b[None, :, None]
    return (xf + o).reshape(x.shape).astype(np.float32)


def kernel(x, block_types, gn_w, gn_b, qkv_w, qkv_b, proj_w, proj_b,
           is_air, is_wood, is_leaves):
    import ml_dtypes
    BF = ml_dtypes.bfloat16

    x = np.ascontiguousarray(np.asarray(x, dtype=np.float32))
    gn_w = np.asarray(gn_w, np.float32); gn_b = np.asarray(gn_b, np.float32)
    qkv_w = np.asarray(qkv_w, np.float32); qkv_b = np.asarray(qkv_b, np.float32)
    proj_w = np.asarray(proj_w, np.float32); proj_b = np.asarray(proj_b, np.float32)
    is_air = np.asarray(is_air, np.float32)
    is_wood = np.asarray(is_wood, np.float32)
    is_leaves = np.asarray(is_leaves, np.float32)
    bt = np.asarray(block_types).reshape(N).astype(np.int64)

    x2 = x.reshape(C, N)
    air = is_air[bt]; wood = is_wood[bt]; leaves = is_leaves[bt]
    idx = np.nonzero(air <= 0.0)[0]
    nk = len(idx)
    if nk == 0:
        return _numpy_reference(x, block_types, gn_w, gn_b, qkv_w, qkv_b,
                                proj_w, proj_b, is_air, is_wood, is_leaves)

    nk_pad = ((nk + 127) // 128) * 128
    nkc = nk_pad // 128
    idx_pad = np.concatenate([idx, np.full(nk_pad - nk, idx[0], np.int64)])

    # --- host-side O(N) feature prep ---
    ypos = ((np.arange(N) // W_) % H_).astype(np.int64)
    oneh = np.zeros((N, 16), np.float32); oneh[np.arange(N), ypos] = 1.0
    m16 = (np.abs(np.arange(16)[:, None] - np.arange(16)[None, :]) <= 2).astype(np.float32)
    lfeat = np.concatenate(
        [(2.0 * wood)[None], (2.0 * leaves)[None], 1.5 * wood[None] * oneh.T]
    ).astype(BF)                                            # [18, N]
    wood_k = wood[idx_pad]; leaves_k = leaves[idx_pad]
    mk = m16 @ oneh[idx_pad].T                              # [16, nk_pad]
    rfeat = np.concatenate(
        [wood_k[None], leaves_k[None], wood_k[None] * mk]
    ).astype(BF)                                            # [18, nk_pad]

    pad_col = np.zeros(nk_pad, np.float32); pad_col[nk:] = 1.0
    pad_m = np.ascontiguousarray(pad_col.reshape(nkc, 128).T)  # [128, nkc]
    abias = pad_m * NEG
    mab = BMAGIC + (pad_m * DVE_PAD_BIAS) * S15

    # GroupNorm statistics on the host (f32, matches the reference exactly)
    xg = x2.reshape(GROUPS, GSIZE * N)
    mu_g = xg.mean(axis=1)
    var_g = xg.var(axis=1)
    rstd_g = 1.0 / np.sqrt(var_g + EPS)
    mu_ch = np.repeat(mu_g, GSIZE); rstd_ch = np.repeat(rstd_g, GSIZE)
    a_ch = (gn_w * rstd_ch).astype(np.float32)
    b_ch = (gn_b - mu_ch * a_ch).astype(np.float32)

    # f32 const blob: [a_c0, a_c1, b_c0, b_c1, bq, bk | abias | mab]
    A0 = 6; M0 = A0 + nkc; CB32 = M0 + nkc
    scale = HD ** -0.5
    cb32_shared = np.zeros((128, CB32), np.float32)
    cb32_shared[:, 0] = a_ch[0:128]; cb32_shared[:, 1] = a_ch[128:256]
    cb32_shared[:, 2] = b_ch[0:128]; cb32_shared[:, 3] = b_ch[128:256]
    cb32_shared[:, A0:M0] = abias
    cb32_shared[:, M0:CB32] = mab

    BV0 = 192; ON0 = 224; CB16 = ON0 + 128

    x2b = np.ascontiguousarray(x2.astype(BF))
    xcb = np.ascontiguousarray(x2[:, idx_pad].astype(BF))

    shared = {
        "x2": x2b, "xc": xcb, "lfeat": np.ascontiguousarray(lfeat),
        "rfeat": np.ascontiguousarray(rfeat),
    }
    in_maps = []
    for hd_i in range(NCORES):
        r0 = hd_i * HD
        cb32_i = cb32_shared.copy()
        cb32_i[0:HD, 4] = qkv_b[0 * C + r0 : 0 * C + r0 + HD] * scale
        cb32_i[0:HD, 5] = qkv_b[1 * C + r0 : 1 * C + r0 + HD]
        cb16_i = np.zeros((128, CB16), np.float32)
        cb16_i[:, 0:HD] = qkv_w[0 * C + r0 : 0 * C + r0 + HD, 0:128].T * scale
        cb16_i[:, HD:2 * HD] = qkv_w[0 * C + r0 : 0 * C + r0 + HD, 128:256].T * scale
        cb16_i[:, 64:64 + HD] = qkv_w[1 * C + r0 : 1 * C + r0 + HD, 0:128].T
        cb16_i[:, 64 + HD:128] = qkv_w[1 * C + r0 : 1 * C + r0 + HD, 128:256].T
        cb16_i[:, 128:128 + HD] = qkv_w[2 * C + r0 : 2 * C + r0 + HD, 0:128].T
        cb16_i[:, 128 + HD:192] = qkv_w[2 * C + r0 : 2 * C + r0 + HD, 128:256].T
        cb16_i[0, BV0:BV0 + HD] = qkv_b[2 * C + r0 : 2 * C + r0 + HD]
        cb16_i[0, ON0:CB16] = 1.0
        m = dict(shared)
        m["cb32"] = np.ascontiguousarray(cb32_i)
        m["cb16"] = np.ascontiguousarray(cb16_i.astype(BF))
        in_maps.append(m)

    if nk_pad not in _CACHE:
        _CACHE[nk_pad] = _build(nk_pad)
    nc = _CACHE[nk_pad]

    use_trace = TRACE
    if use_trace:
        import importlib.util
        if importlib.util.find_spec("antenv.axon_hooks") is None:
            use_trace = False
    res = run_bass_kernel_spmd(nc, in_maps, core_ids=list(range(NCORES)), trace=use_trace)
    LAST_RESULT["res"] = res

    # host: normalize + projection + residual
    attn_all = np.empty((N, C), np.float32)
    for i in range(NCORES):
        o = np.asarray(res.results[i]["o"], np.float32)        # [4, 128, 264]
        oh = o.reshape(NQG, 128, 8, HD + 1).transpose(0, 2, 1, 3).reshape(N, HD + 1)
        attn_all[:, i * HD : (i + 1) * HD] = oh[:, 0:HD] / oh[:, HD : HD + 1]
    y = x2 + proj_w @ attn_all.T.astype(np.float32) + proj_b[:, None]
    return y.reshape(B, C, D_, H_, W_).astype(np.float32)


# revision 54
# speedup vs baseline: 1.5849x; 1.0102x over previous
"""Trainium2 Bass kernel for MinecraftAwareAttention3D (v2).

Full (unsharded) inputs in, full output out. One attention head per NeuronCore
(tensor parallel over num_heads=8); GroupNorm + QKV replicated per core;
per-head softmax numerator/denominator returned to the host, which applies the
normalize + output projection + residual (cheap 4096x256x256 sgemm).

Key structure (chosen against the TimelineSim cost model):
  * All activations bf16: halves input DMA, enables 1-cycle/row PE matmuls
    at any output width and DVE 2x/4x modes for the normalize.
  * Boosts folded into QK^T as 18 extra bf16 contraction rows; air keys
    compacted away on the host (nk_pad ~3072 instead of 4096).
  * exp of the score matrix is split between ACT (hardware Exp, bf16 out)
    and DVE (Schraudolph bit-trick: t = s*S15 + MAGIC; bits<<8 = exp bits;
    f32->bf16 narrowing copy on the idle GpSimd/Pool engine).
  * Transposed PV: out[128q, 33] += pt[128k,128q]^T @ [v|1][128k, 33] per
    key chunk -- full 128-partition output, 33-row bf16 matmuls, and the
    softmax denominator rides along as column 32.
  * V^T built directly by transposed-V matmuls (contraction over channels),
    with the v-bias added via a rank-1 ones-row matmul.
  * No normalize / projection on device: the host divides by the
    denominator and does the 256x256 projection fused with the residual.
"""

import numpy as np

import concourse.bass as bass
import concourse.tile as tile
from concourse import mybir
from concourse.bass_utils import run_bass_kernel_spmd

F32 = mybir.dt.float32
F32R = mybir.dt.float32r
BF16 = mybir.dt.bfloat16
I32 = mybir.dt.int32
AF = mybir.ActivationFunctionType
ALU = mybir.AluOpType

B, C, D_, H_, W_ = 1, 256, 16, 16, 16
N = D_ * H_ * W_          # 4096 spatial positions
HEADS, HD = 8, 32
GROUPS = 8
GSIZE = C // GROUPS
EPS = 1e-5
NEG = -1e9
NF = HD + 18              # fused contraction depth: 32 qk dims + 18 boost dims
NCORES = 8
NQG = N // 1024           # 4 query groups of 1024

# Schraudolph fast-exp constants (2^15 scaling, magic 2^23)
S15 = float((1 << 15) * 1.4426950408889634)
BMAGIC = float((127.0 - 0.0437) * (1 << 15) + (1 << 23))
DVE_PAD_BIAS = -60.0      # pad-key logit bias on DVE chunks (exp ~ e^-60)
DVE_FRAC_NUM, DVE_FRAC_DEN = 7, 24   # ~7/24 of key chunks take the DVE path

TRACE = False             # test.py can flip this for profiling
LAST_RESULT = {}

_CACHE = {}


def _dve_chunk(kc, nkc, qg=1):
    """Evenly spread DVE-assigned key chunks among ACT ones, keeping the
    first chunks and the last three on ACT (ramp/tail latency). In the
    first q-group DVE is still draining phase-1/2 normalizes, so its DVE
    chunks start later."""
    n_dve = (nkc * DVE_FRAC_NUM) // DVE_FRAC_DEN
    if qg == 0:
        n_dve = (nkc * 5) // 24
    lo = 2 if qg == 0 else 1
    if kc < lo or kc >= nkc - 5:
        return False
    j, m = kc - lo, nkc - 3 - lo
    return ((j + 1) * n_dve) // m > (j * n_dve) // m


def _split_waits(nc, max_waits=1):
    """This walrus build only encodes one sync wait per instruction; hoist
    extra waits onto same-engine NOPs inserted just before the instruction."""
    n = 0
    for f in nc.m.functions:
        for bb in f.blocks:
            new_insts = []
            for inst in bb.instructions:
                si = inst.sync_info
                if si is not None and si.on_wait and len(si.on_wait) > max_waits:
                    waits = list(si.on_wait)
                    si.on_wait = waits[-max_waits:]
                    for i in range(0, len(waits) - max_waits, max_waits):
                        n += 1
                        nop = mybir.InstNoOp(name=f"I-wsplit-{n}", ins=[], outs=[])
                        nop.engine = inst.engine
                        nop.sync_info = mybir.SyncInfo(
                            on_wait=waits[i : i + max_waits], on_update=[]
                        )
                        new_insts.append(nop)
                new_insts.append(inst)
            bb.instructions[:] = new_insts
    return n


def _build(nk_pad):
    """Build the per-core Bass module; static on the padded compacted key
    count. All data arrives as ExternalInputs."""
    nkc = nk_pad // 128                       # 128-key chunks
    nks = (nk_pad + 1023) // 1024             # 1024-col xc slices per half
    kslices = [(s, min(s + 512, nk_pad)) for s in range(0, nk_pad, 512)]

    # f32 const-blob column layout: [a_c0, a_c1, b_c0, b_c1, bq, bk | abias | mab]
    A0 = 6                    # abias cols
    M0 = A0 + nkc             # mab cols
    CB32 = M0 + nkc
    # bf16 const-blob column layout
    BV0 = 192                 # bvT row (row 0)
    ON0 = 224                 # ones row (row 0)
    CB16 = ON0 + 128

    nc = bass.Bass()

    # ---- I/O ----
    x2 = nc.dram_tensor("x2", [C, N], BF16, kind="ExternalInput")
    xc = nc.dram_tensor("xc", [C, nk_pad], BF16, kind="ExternalInput")
    lfeat = nc.dram_tensor("lfeat", [18, N], BF16, kind="ExternalInput")
    rfeat = nc.dram_tensor("rfeat", [18, nk_pad], BF16, kind="ExternalInput")
    cb32 = nc.dram_tensor("cb32", [128, CB32], F32, kind="ExternalInput")
    cb16 = nc.dram_tensor("cb16", [128, CB16], BF16, kind="ExternalInput")
    out = nc.dram_tensor("o", [NQG, 128, 264], F32, kind="ExternalOutput")

    with tile.TileContext(nc) as tc:
        with (
            tc.tile_pool(name="consts", bufs=1) as cp,
            tc.tile_pool(name="live", bufs=1) as lp,
            tc.tile_pool(name="small", bufs=2) as sp,
            tc.tile_pool(name="ptpool", bufs=7) as ptp,
            tc.tile_pool(name="tpool", bufs=2) as tp_,
            tc.tile_pool(name="opool", bufs=2) as op,
            tc.tile_pool(name="ps_qkv", bufs=2, space="PSUM") as ps_qkv,
            tc.tile_pool(name="ps_st", bufs=2, space="PSUM") as ps_st,
            tc.tile_pool(name="ps_std", bufs=1, space="PSUM") as ps_std,
            tc.tile_pool(name="ps_pv", bufs=1, space="PSUM") as ps_pv,
        ):
            # ---- long-lived activations ----
            h = [lp.tile([128, N], BF16, name=f"h{c}") for c in range(2)]
            hk = [lp.tile([128, nk_pad], BF16, name=f"hk{c}") for c in range(2)]
            qf = lp.tile([NF, N], BF16)           # Q' = [q*scale ; L]
            kf = lp.tile([NF, nk_pad], BF16)      # K' = [k ; R]
            vt = lp.tile([128, nkc, HD + 1], BF16)  # per-chunk [v ; 1]^T

            # Warm the ACT exp table-set before anything else touches ACT.
            wz = cp.tile([1, 1], F32)
            nc.vector.memset(wz, 0.0)
            wy = cp.tile([1, 1], F32)
            nc.scalar.activation(out=wy, in_=wz, func=AF.Exp, bias=0.0, scale=1.0)
            # zero PE weights: opens each q-group's PSUM accumulation region
            # with a single spanning matmul (a region-sliced start=True
            # clobbers sibling regions in the same PSUM bank on hardware)
            zw = cp.tile([128, 128], BF16)
            nc.vector.memset(zw, 0.0)

            # ================= Phase 1: loads + GroupNorm =================
            # GroupNorm statistics are computed on the host (pure function of
            # the input); the device only applies h = a*x + b. DMA order puts
            # the weights and the first x2/xc slices first so the first QK
            # chunk is ready ~10us in.
            with tc.tile_pool(name="xpool", bufs=1) as xp:
                xt = [xp.tile([128, N], BF16, name=f"xt{c}") for c in range(2)]
                for c in range(2):
                    nc.sync.dma_start(
                        out=xt[c][:, 0:512],
                        in_=x2[c * 128 : (c + 1) * 128, 0:512],
                    )
                cb16_t = cp.tile([128, CB16], BF16)
                nc.sync.dma_start(out=cb16_t, in_=cb16[:, :])
                cb32_t = cp.tile([128, CB32], F32)
                nc.sync.dma_start(out=cb32_t, in_=cb32[:, :])
                xcs = []
                for s in range(nks):
                    s0, s1 = s * 1024, min((s + 1) * 1024, nk_pad)
                    pair = []
                    for c in range(2):
                        xs_t = xp.tile(
                            [128, 1024], BF16, name="xcs", tag="xcs", bufs=2 * nks
                        )
                        pair.append(xs_t)
                    xcs.append(pair)
                for c in range(2):
                    nc.sync.dma_start(
                        out=xcs[0][c][:, 0:512],
                        in_=xc[c * 128 : (c + 1) * 128, 0:512],
                    )
                for c in range(2):
                    nc.sync.dma_start(
                        out=xt[c][:, 512:2048],
                        in_=x2[c * 128 : (c + 1) * 128, 512:2048],
                    )
                for c in range(2):
                    nc.sync.dma_start(
                        out=xcs[0][c][:, 512:1024],
                        in_=xc[c * 128 : (c + 1) * 128, 512:1024],
                    )
                nc.sync.dma_start(out=qf[HD:NF, :], in_=lfeat[:, :])
                nc.sync.dma_start(out=kf[HD:NF, :], in_=rfeat[:, :])
                for c in range(2):
                    nc.sync.dma_start(
                        out=xt[c][:, 2048:N],
                        in_=x2[c * 128 : (c + 1) * 128, 2048:N],
                    )
                for s in range(1, nks):
                    s0, s1 = s * 1024, min((s + 1) * 1024, nk_pad)
                    for c in range(2):
                        nc.sync.dma_start(
                            out=xcs[s][c][:, 0 : s1 - s0],
                            in_=xc[c * 128 : (c + 1) * 128, s0:s1],
                        )

                # ones column of V'T
                nc.gpsimd.memset(vt[:, :, HD : HD + 1], 1.0)

                ab = [(cb32_t[:, c : c + 1], cb32_t[:, 2 + c : 3 + c]) for c in range(2)]

                # queries: first 512 columns first (unblocks the first QK),
                # then the rest of the first half; key slice 0. Later hk
                # slices and the h second half are emitted just-in-time in
                # the qg0 loop to keep the DVE FIFO unblocked.
                for c in range(2):
                    a_ch, b_ch = ab[c]
                    nc.vector.tensor_scalar(
                        out=h[c][:, 0:512], in0=xt[c][:, 0:512],
                        scalar1=a_ch, scalar2=b_ch, op0=ALU.mult, op1=ALU.add,
                    )
                for c in range(2):
                    a_ch, b_ch = ab[c]
                    nc.vector.tensor_scalar(
                        out=h[c][:, 512:2048], in0=xt[c][:, 512:2048],
                        scalar1=a_ch, scalar2=b_ch, op0=ALU.mult, op1=ALU.add,
                    )

                emitted_hk = set()
                emitted_hrest = [False]

                def emit_hk(s, parts=1):
                    if s in emitted_hk or s >= nks:
                        return
                    emitted_hk.add(s)
                    s0, s1 = s * 1024, min((s + 1) * 1024, nk_pad)
                    bounds = [s0 + (s1 - s0) * i // parts for i in range(parts + 1)]
                    for p in range(parts):
                        for c in range(2):
                            a_ch, b_ch = ab[c]
                            nc.vector.tensor_scalar(
                                out=hk[c][:, bounds[p] : bounds[p + 1]],
                                in0=xcs[s][c][:, bounds[p] - s0 : bounds[p + 1] - s0],
                                scalar1=a_ch, scalar2=b_ch,
                                op0=ALU.mult, op1=ALU.add,
                            )

                def emit_hrest():
                    if emitted_hrest[0]:
                        return
                    emitted_hrest[0] = True
                    for c in range(2):
                        a_ch, b_ch = ab[c]
                        nc.vector.tensor_scalar(
                            out=h[c][:, 2048:N], in0=xt[c][:, 2048:N],
                            scalar1=a_ch, scalar2=b_ch, op0=ALU.mult, op1=ALU.add,
                        )

                emit_hk(0, parts=2)

            # ========== Phase 2+3: QKV emission fused into attention ==========
            # Phase-2 work (K slices, Q slices, V^T chunks) is emitted
            # just-in-time inside the first q-group's chunk loop so the PE
            # FIFO never parks early QK matmuls behind V^T chunks that wait
            # on late xc DMA slices.
            emitted_k = set()
            emitted_q = set()
            emitted_vt = set()

            evac_rr = [0]

            def _evacuate(dst_ap, ps_ap, bias_ap):
                # Round-robin the PSUM->SBUF bias-evacuation across ACT/DVE/
                # Pool so consecutive QKV slices pipeline instead of
                # serializing behind one engine's FIFO.
                e = evac_rr[0] % 2
                evac_rr[0] += 1
                if e == 0:
                    nc.scalar.add(out=dst_ap, in_=ps_ap, add=bias_ap)
                else:
                    nc.vector.tensor_scalar_add(out=dst_ap, in0=ps_ap, scalar1=bias_ap)

            def emit_k(j, act=False):
                if j in emitted_k or j >= len(kslices):
                    return
                emitted_k.add(j)
                s0, s1 = kslices[j]
                ps = ps_qkv.tile([128, 512], F32, space="PSUM", name="qkv_ps", tag="s")
                for c in range(2):
                    nc.tensor.matmul(
                        ps[0:HD, 0 : s1 - s0],
                        lhsT=cb16_t[:, 64 + c * HD : 64 + (c + 1) * HD],
                        rhs=hk[c][:, s0:s1],
                        start=(c == 0),
                        stop=(c == 1),
                    )
                _evacuate(kf[0:HD, s0:s1], ps[0:HD, 0 : s1 - s0], cb32_t[0:HD, 5:6])

            def emit_q(i, act=False):
                if i in emitted_q or i >= 8:
                    return
                emitted_q.add(i)
                q0 = i * 512
                ps = ps_qkv.tile([128, 512], F32, space="PSUM", name="qkv_ps", tag="s")
                for c in range(2):
                    nc.tensor.matmul(
                        ps[0:HD, :],
                        lhsT=cb16_t[:, c * HD : (c + 1) * HD],
                        rhs=h[c][:, q0 : q0 + 512],
                        start=(c == 0),
                        stop=(c == 1),
                    )
                _evacuate(qf[0:HD, q0 : q0 + 512], ps[0:HD, :], cb32_t[0:HD, 4:5])

            def emit_vt(kc):
                if kc in emitted_vt or kc >= nkc:
                    return
                emitted_vt.add(kc)
                k0 = kc * 128
                tps = ps_qkv.tile([128, 512], F32, space="PSUM", name="qkv_ps", tag="s")
                for c in range(2):
                    nc.tensor.matmul(
                        tps[:, 0:HD],
                        lhsT=hk[c][:, k0 : k0 + 128],
                        rhs=cb16_t[:, 128 + c * HD : 128 + (c + 1) * HD],
                        start=(c == 0),
                        stop=False,
                    )
                nc.tensor.matmul(
                    tps[:, 0:HD],
                    lhsT=cb16_t[0:1, ON0 : ON0 + 128],
                    rhs=cb16_t[0:1, BV0 : BV0 + HD],
                    start=False,
                    stop=True,
                )
                if kc % 2 == 0:
                    nc.scalar.activation(
                        out=vt[:, kc, 0:HD], in_=tps[:, 0:HD], func=AF.Copy,
                    )
                else:
                    nc.vector.tensor_copy(out=vt[:, kc, 0:HD], in_=tps[:, 0:HD])

            emit_k(0, act=True)
            emit_q(0, act=True)
            emit_q(1, act=True)

            def emit_pvt(kc, pvq, pt, vlhs):
                for qb in range(8):
                    nc.tensor.matmul(
                        pvq[:, qb * (HD + 1) : (qb + 1) * (HD + 1)],
                        lhsT=pt[:, qb * 128 : (qb + 1) * 128],
                        rhs=vlhs,
                        start=False,
                        stop=(kc == nkc - 1),
                        skip_group_check=True,
                    )

            if True:
                total = NQG * nkc
                sts = {}
                pvqs = {}
                pending_pvt = []

                def issue_qk(g):
                    if g >= total:
                        return
                    qg, kc = divmod(g, nkc)
                    q0 = qg * 1024
                    emit_k(kc // 4)
                    lhs = kf[:, kc * 128 : (kc + 1) * 128]
                    if _dve_chunk(kc, nkc, qg):
                        sta = ps_std.tile([128, 512], F32, space="PSUM", name="std")
                        if qg >= 1:
                            # qkv pool is idle after qg0; avoids serializing
                            # the two halves through the single std buffer
                            stb = ps_qkv.tile(
                                [128, 512], F32, space="PSUM", name="qkv_ps", tag="s"
                            )
                        else:
                            stb = ps_std.tile([128, 512], F32, space="PSUM", name="std")
                        nc.tensor.matmul(
                            sta, lhsT=lhs, rhs=qf[:, q0 : q0 + 512],
                            start=True, stop=True,
                        )
                        nc.tensor.matmul(
                            stb, lhsT=lhs, rhs=qf[:, q0 + 512 : q0 + 1024],
                            start=True, stop=True,
                        )
                        sts[g] = (sta, stb)
                    else:
                        st = ps_st.tile([128, 1024], F32, space="PSUM", name="st")
                        nc.tensor.matmul(
                            st[:, 0:512], lhsT=lhs, rhs=qf[:, q0 : q0 + 512],
                            start=True, stop=True,
                        )
                        nc.tensor.matmul(
                            st[:, 512:1024], lhsT=lhs,
                            rhs=qf[:, q0 + 512 : q0 + 1024],
                            start=True, stop=True,
                        )
                        sts[g] = st

                def get_pvq(qg):
                    if qg not in pvqs:
                        pvq = ps_pv.tile(
                            [128, 8 * (HD + 1)], F32, space="PSUM", name="pvq"
                        )
                        nc.tensor.matmul(
                            pvq, lhsT=zw, rhs=cb16_t[:, 0 : 8 * (HD + 1)],
                            start=True, stop=False, skip_group_check=True,
                        )
                        pvqs[qg] = pvq
                    return pvqs[qg]

                def flush_pvt(up_to=None):
                    while pending_pvt and (up_to is None or pending_pvt[0][0] <= up_to):
                        g2, pt2 = pending_pvt.pop(0)
                        qg2, kc2 = divmod(g2, nkc)
                        emit_pvt(kc2, get_pvq(qg2), pt2, vt[:, kc2, :])
                        if kc2 == nkc - 1:
                            finish_qg(qg2)

                def finish_qg(qg2):
                    hp = tc.high_priority()
                    hp.__enter__()
                    ot = op.tile([128, 8 * (HD + 1)], F32, name="ot")
                    nc.vector.tensor_copy(out=ot, in_=pvqs.pop(qg2))
                    nc.sync.dma_start(out=out[qg2, :, :], in_=ot)
                    hp.__exit__(None, None, None)

                issue_qk(0)
                issue_qk(1)
                for g in range(total):
                    qg, kc = divmod(g, nkc)
                    st = sts.pop(g)
                    pt = ptp.tile([128, 1024], BF16, name="pt")
                    if _dve_chunk(kc, nkc, qg):
                        # Schraudolph fast-exp on DVE + narrowing on Pool
                        sta, stb = st
                        t = tp_.tile([128, 1024], F32, name="t")
                        for hf, sth in ((0, sta), (1, stb)):
                            nc.vector.tensor_scalar(
                                out=t[:, hf * 512 : (hf + 1) * 512], in0=sth,
                                scalar1=S15,
                                scalar2=cb32_t[:, M0 + kc : M0 + kc + 1],
                                op0=ALU.mult, op1=ALU.add,
                            )
                        nc.vector.tensor_scalar(
                            out=t.bitcast(I32), in0=t.bitcast(I32),
                            scalar1=8, scalar2=None,
                            op0=ALU.logical_shift_left,
                        )
                        nc.gpsimd.tensor_copy(out=pt, in_=t)
                    else:
                        nc.scalar.activation(
                            out=pt, in_=st, func=AF.Exp,
                            bias=cb32_t[:, A0 + kc : A0 + kc + 1], scale=1.0,
                        )
                    if qg == 0:
                        emit_vt(kc + 2)
                        if kc == 2:
                            emit_hk(1)
                        if kc == 6:
                            emit_hk(2)
                            emit_hrest()
                        if kc >= 3:
                            emit_q(2 + (kc - 3) // 3)
                    if g == nkc - 3:
                        for j in range(len(kslices)):
                            emit_k(j)
                        for i in range(8):
                            emit_q(i)
                        for kc2 in range(nkc):
                            emit_vt(kc2)
                    issue_qk(g + 2)
                    flush_pvt(up_to=g - 6)
                    pending_pvt.append((g, pt))
                flush_pvt()

    _split_waits(nc)
    return nc


def _numpy_reference(x, block_types, gn_w, gn_b, qkv_w, qkv_b, proj_w, proj_b,
                     is_air, is_wood, is_leaves):
    """Pure-numpy fallback (degenerate case: no non-air keys)."""
    xf = x.reshape(B, C, N).astype(np.float64)
    xs = xf.reshape(B, GROUPS, GSIZE * N)
    mu = xs.mean(axis=2, keepdims=True)
    var = xs.var(axis=2, keepdims=True)
    hh = ((xs - mu) / np.sqrt(var + EPS)).reshape(B, C, N)
    hh = hh * gn_w[None, :, None] + gn_b[None, :, None]
    qkv = np.einsum("oc,bcn->bon", qkv_w.astype(np.float64), hh) + qkv_b[None, :, None]
    qkv = qkv.reshape(B, 3, HEADS, HD, N)
    q, k, v = qkv[:, 0], qkv[:, 1], qkv[:, 2]
    attn = np.einsum("bhdn,bhdm->bhnm", q, k) * (HD ** -0.5)
    bf = block_types.reshape(B, N)
    air = is_air[bf]; wood = is_wood[bf]; leaves = is_leaves[bf]
    attn = np.where(air[:, None, None, :] > 0, NEG, attn)
    wo = wood[:, :, None] * wood[:, None, :]
    lo = leaves[:, :, None] * leaves[:, None, :]
    mb = np.clip((wo + lo) * 2.0, 0.0, 10.0)
    pos = np.arange(N); ypos = (pos // W_) % H_
    vm = (np.abs(ypos[None, :] - ypos[:, None]) <= 2).astype(np.float64)
    vb = np.clip(wo * vm[None] * 1.5, 0.0, 10.0)
    attn = attn + (mb + vb)[:, None]
    attn = attn - attn.max(axis=-1, keepdims=True)
    e = np.exp(attn); p = e / e.sum(axis=-1, keepdims=True)
    o = np.einsum("bhnm,bhdm->bhdn", p, v).reshape(B, C, N)
    o = np.einsum("oc,bcn->bon", proj_w.astype(np.float64), o) + proj_b[None, :, None]
    return (xf + o).reshape(x.shape).astype(np.float32)


def kernel(x, block_types, gn_w, gn_b, qkv_w, qkv_b, proj_w, proj_b,
           is_air, is_wood, is_leaves):
    import ml_dtypes
    BF = ml_dtypes.bfloat16

    x = np.ascontiguousarray(np.asarray(x, dtype=np.float32))
    gn_w = np.asarray(gn_w, np.float32); gn_b = np.asarray(gn_b, np.float32)
    qkv_w = np.asarray(qkv_w, np.float32); qkv_b = np.asarray(qkv_b, np.float32)
    proj_w = np.asarray(proj_w, np.float32); proj_b = np.asarray(proj_b, np.float32)
    is_air = np.asarray(is_air, np.float32)
    is_wood = np.asarray(is_wood, np.float32)
    is_leaves = np.asarray(is_leaves, np.float32)
    bt = np.asarray(block_types).reshape(N).astype(np.int64)

    x2 = x.reshape(C, N)
    air = is_air[bt]; wood = is_wood[bt]; leaves = is_leaves[bt]
    idx = np.nonzero(air <= 0.0)[0]
    nk = len(idx)
    if nk == 0:
        return _numpy_reference(x, block_types, gn_w, gn_b, qkv_w, qkv_b,
                                proj_w, proj_b, is_air, is_wood, is_leaves)

    nk_pad = ((nk + 127) // 128) * 128
    nkc = nk_pad // 128
    idx_pad = np.concatenate([idx, np.full(nk_pad - nk, idx[0], np.int64)])

    # --- host-side O(N) feature prep ---
    ypos = ((np.arange(N) // W_) % H_).astype(np.int64)
    oneh = np.zeros((N, 16), np.float32); oneh[np.arange(N), ypos] = 1.0
    m16 = (np.abs(np.arange(16)[:, None] - np.arange(16)[None, :]) <= 2).astype(np.float32)
    lfeat = np.concatenate(
        [(2.0 * wood)[None], (2.0 * leaves)[None], 1.5 * wood[None] * oneh.T]
    ).astype(BF)                                            # [18, N]
    wood_k = wood[idx_pad]; leaves_k = leaves[idx_pad]
    mk = m16 @ oneh[idx_pad].T                              # [16, nk_pad]
    rfeat = np.concatenate(
        [wood_k[None], leaves_k[None], wood_k[None] * mk]
    ).astype(BF)                                            # [18, nk_pad]

    pad_col = np.zeros(nk_pad, np.float32); pad_col[nk:] = 1.0
    pad_m = np.ascontiguousarray(pad_col.reshape(nkc, 128).T)  # [128, nkc]
    abias = pad_m * NEG
    mab = BMAGIC + (pad_m * DVE_PAD_BIAS) * S15

    # GroupNorm statistics on the host (f32, matches the reference exactly)
    xg = x2.reshape(GROUPS, GSIZE * N)
    mu_g = xg.mean(axis=1)
    var_g = xg.var(axis=1)
    rstd_g = 1.0 / np.sqrt(var_g + EPS)
    mu_ch = np.repeat(mu_g, GSIZE); rstd_ch = np.repeat(rstd_g, GSIZE)
    a_ch = (gn_w * rstd_ch).astype(np.float32)
    b_ch = (gn_b - mu_ch * a_ch).astype(np.float32)

    # f32 const blob: [a_c0, a_c1, b_c0, b_c1, bq, bk | abias | mab]
    A0 = 6; M0 = A0 + nkc; CB32 = M0 + nkc
    scale = HD ** -0.5
    cb32_shared = np.zeros((128, CB32), np.float32)
    cb32_shared[:, 0] = a_ch[0:128]; cb32_shared[:, 1] = a_ch[128:256]
    cb32_shared[:, 2] = b_ch[0:128]; cb32_shared[:, 3] = b_ch[128:256]
    cb32_shared[:, A0:M0] = abias
    cb32_shared[:, M0:CB32] = mab

    BV0 = 192; ON0 = 224; CB16 = ON0 + 128

    x2b = np.ascontiguousarray(x2.astype(BF))
    xcb = np.ascontiguousarray(x2[:, idx_pad].astype(BF))

    shared = {
        "x2": x2b, "xc": xcb, "lfeat": np.ascontiguousarray(lfeat),
        "rfeat": np.ascontiguousarray(rfeat),
    }
    in_maps = []
    for hd_i in range(NCORES):
        r0 = hd_i * HD
        cb32_i = cb32_shared.copy()
        cb32_i[0:HD, 4] = qkv_b[0 * C + r0 : 0 * C + r0 + HD] * scale
        cb32_i[0:HD, 5] = qkv_b[1 * C + r0 : 1 * C + r0 + HD]
        cb16_i = np.zeros((128, CB16), np.float32)
        cb16_i[:, 0:HD] = qkv_w[0 * C + r0 : 0 * C + r0 + HD, 0:128].T * scale
        cb16_i[:, HD:2 * HD] = qkv_w[0 * C + r0 : 0 * C + r0 + HD, 128:256].T * scale
        cb16_i[:, 64:64 + HD] = qkv_w[1 * C + r0 : 1 * C + r0 + HD, 0:128].T
        cb16_i[:, 64 + HD:128] = qkv_w[1 * C + r0 : 1 * C + r0 + HD, 128:256].T
        cb16_i[:, 128:128 + HD] = qkv_w[2 * C + r0 : 2 * C + r0 + HD, 0:128].T
        cb16_i[:, 128 + HD:192] = qkv_w[2 * C + r0 : 2 * C + r0 + HD, 128:256].T
        cb16_i[0, BV0:BV0 + HD] = qkv_b[2 * C + r0 : 2 * C + r0 + HD]
        cb16_i[0, ON0:CB16] = 1.0
        m = dict(shared)
        m["cb32"] = np.ascontiguousarray(cb32_i)
        m["cb16"] = np.ascontiguousarray(cb16_i.astype(BF))
        in_maps.append(m)

    if nk_pad not in _CACHE:
        _CACHE[nk_pad] = _build(nk_pad)
    nc = _CACHE[nk_pad]

    use_trace = TRACE
    if use_trace:
        import importlib.util
        if importlib.util.find_spec("antenv.axon_hooks") is None:
            use_trace = False
    res = run_bass_kernel_spmd(nc, in_maps, core_ids=list(range(NCORES)), trace=use_trace)
    LAST_RESULT["res"] = res

    # host: normalize + projection + residual
    attn_all = np.empty((N, C), np.float32)
    for i in range(NCORES):
        o = np.asarray(res.results[i]["o"], np.float32)        # [4, 128, 264]
        oh = o.reshape(NQG, 128, 8, HD + 1).transpose(0, 2, 1, 3).reshape(N, HD + 1)
        attn_all[:, i * HD : (i + 1) * HD] = oh[:, 0:HD] / oh[:, HD : HD + 1]
    y = x2 + proj_w @ attn_all.T.astype(np.float32) + proj_b[:, None]
    return y.reshape(B, C, D_, H_, W_).astype(np.float32)
